# revision 9
# baseline (speedup 1.0000x reference)
"""Trainium2 Bass kernel for a full causal MHA layer (B=2, T=2048, C=2048, H=16,
partial RoPE on first 64 dims of each 128-dim head).

Sharding over 8 cores: core c handles batch b=c//4 and heads [4g, 4g+4), g=c%4
(tensor-parallel over heads x data-parallel over batch).

Fully fused single pass per core, fp16 data plane (fp32 PSUM accumulation):
  for each 512-token chunk ic:
    proj q/k (fp16 weights stationary, fp16 x moving), bias+partial-RoPE,
      q and k stay resident in SBUF (no DRAM spills)
    proj v -> v_res [key, jt, m] fp16 resident
    attention for chunk ic over heads h: per key-tile jt
      scoresT[k,q] (k_res stationary fp16, q moving fp16)
      -> exp(scale*s - 10*ln2) -> ex fp16 (Act), triangle mask on diagonal
         tiles only (gpsimd affine_select), exact causal col-trimming
      -> av accumulation outT[d,q] via PE; softmax denominator via DVE
         exsum adds + gpsimd partition_all_reduce (no PE ones-matmuls)
      output-projection matmuls of chunk ic-1 are woven between attention
      matmuls to keep PE busy during Act-latency windows
    phase3(ic): out partial outT[c,q] = sum_mt woT attn, DVE evict fp16, DMA
Host: slices inputs per core (fp16), sums the 4 TP partials per batch + bo.
"""

import math

import numpy as np

import concourse.bass as bass
import concourse.bass_isa as bass_isa
import concourse.mybir as mybir
import concourse.tile as tile
from concourse import bacc
from concourse.bass_utils import run_bass_kernel_spmd

F32 = mybir.dt.float32
F16 = mybir.dt.float16

B, T, C = 2, 2048, 2048
H = 16
HS = 128
ROT = 64
HALF = 32
BASE = 10000.0

N_CORES = 8
TPG = 4                # TP group size (heads split)
H_LOC = H // TPG       # 4 heads per core
M = H_LOC * HS         # 512 local head-dim columns
SCALE = 1.0 / math.sqrt(HS)
EXP_BIAS = -10.0 * math.log(2.0)   # exp(s*SCALE - 10ln2): keeps fp16 in range

P = 128
NT = T // 512          # 4 t-chunks of 512
CT = C // P            # 16 contraction tiles
JT = T // P            # 16 key tiles per head

_NC_CACHE = {}


def _build(phases=(1, 2, 3)):
    nc = bacc.Bacc(None, target_bir_lowering=False)

    xh = nc.declare_dram_parameter("xh", [C, T], F16, isOutput=False)
    wqT = nc.declare_dram_parameter("wqT", [C, M], F16, isOutput=False)
    wkT = nc.declare_dram_parameter("wkT", [C, M], F16, isOutput=False)
    wvT = nc.declare_dram_parameter("wvT", [C, M], F16, isOutput=False)
    woT = nc.declare_dram_parameter("woT", [M, C], F16, isOutput=False)
    bqc = nc.declare_dram_parameter("bqc", [P, H_LOC], F32, isOutput=False)
    bkc = nc.declare_dram_parameter("bkc", [P, H_LOC], F32, isOutput=False)
    bvr = nc.declare_dram_parameter("bvr", [1, M], F32, isOutput=False)
    ebias = nc.declare_dram_parameter("ebias", [P, 1], F32, isOutput=False)
    trimask = nc.declare_dram_parameter("trimask", [P, P], F16, isOutput=False)
    cosT = nc.declare_dram_parameter("cosT", [ROT, T], F16, isOutput=False)
    nsT = nc.declare_dram_parameter("nsT", [ROT, T], F16, isOutput=False)
    outT = nc.declare_dram_parameter("outT", [C, T], F16, isOutput=True)

    with tile.TileContext(nc) as tc, \
         tc.tile_pool(name="const", bufs=1) as const, \
         tc.tile_pool(name="wp", bufs=CT) as wpool, \
         tc.tile_pool(name="wop", bufs=H_LOC) as wopool, \
         tc.tile_pool(name="xp", bufs=CT) as xpool, \
         tc.tile_pool(name="qc", bufs=2) as qpool, \
         tc.tile_pool(name="at", bufs=2) as atpool, \
         tc.tile_pool(name="rp", bufs=6) as rpool, \
         tc.tile_pool(name="exp", bufs=6) as expool, \
         tc.tile_pool(name="exs", bufs=2) as espool, \
         tc.tile_pool(name="rd", bufs=2) as rdpool, \
         tc.tile_pool(name="oe", bufs=3) as oepool, \
         tc.tile_pool(name="psA", bufs=3, space="PSUM") as psA, \
         tc.tile_pool(name="psS", bufs=3, space="PSUM") as psS, \
         tc.tile_pool(name="psV", bufs=2, space="PSUM") as psV:

        cos_sb = const.tile([ROT, T], F16, tag="cos")
        ns_sb = const.tile([ROT, T], F16, tag="ns")
        bq_sb = const.tile([P, H_LOC], F32, tag="bq")
        bk_sb = const.tile([P, H_LOC], F32, tag="bk")
        bv_sb = const.tile([1, M], F32, tag="bv")
        bvb_sb = const.tile([P, M], F32, tag="bvb")
        eb_sb = const.tile([P, 1], F32, tag="ebias")
        tri_sb = const.tile([P, P], F16, tag="trimask")
        k_res = const.tile([P, H_LOC, T], F16, tag="kres")
        v_res = const.tile([P, JT, M], F16, tag="vres")
        wq_b = const.tile([P, CT, M], F16, tag="wqb")
        wk_b = const.tile([P, CT, M], F16, tag="wkb")
        wv_b = const.tile([P, CT, M], F16, tag="wvb")
        wo_b = const.tile([P, H_LOC, C], F16, tag="wob")
        wqr = wqT[:].rearrange("(ct p) m -> p ct m", p=P)
        wkr = wkT[:].rearrange("(ct p) m -> p ct m", p=P)
        wvr = wvT[:].rearrange("(ct p) m -> p ct m", p=P)
        wor = woT[:].rearrange("(mt p) c -> p mt c", p=P)

        xr = xh[:].rearrange("(ct p) t -> p ct t", p=P)
        otr = outT[:].rearrange("(co p) t -> p co t", p=P)

        def load_x(ic):
            ts0 = ic * 512
            xb = xpool.tile([P, CT, 512], F16, tag="xb", name=f"xb{ic}", bufs=2)
            for j in range(4):
                nc.sync.dma_start(out=xb[:, 4 * j:4 * j + 4, :],
                                  in_=xr[:, 4 * j:4 * j + 4, ts0:ts0 + 512])
            return [xb[:, ct, :] for ct in range(CT)]

        # startup: x0 streams on the SP queue while wq streams on the Act
        # queue; chunked so HWDGE fixed overheads don't pace the start.
        xb0 = xpool.tile([P, CT, 512], F16, tag="xb", name="xb0", bufs=2)
        x0_chunks = [(2 * j, 2 * j + 2) for j in range(8)]
        for a, b in x0_chunks:
            nc.sync.dma_start(out=xb0[:, a:b, :], in_=xr[:, a:b, 0:512])
        x0_sl = [xb0[:, ct, :] for ct in range(CT)]
        for a, b in x0_chunks:
            nc.scalar.dma_start(out=wq_b[:, a:b, :], in_=wqr[:, a:b, :])
        for j in range(4):
            nc.sync.dma_start(out=wv_b[:, 4 * j:4 * j + 4, :],
                              in_=wvr[:, 4 * j:4 * j + 4, :])
        for j in range(4):
            nc.sync.dma_start(out=wk_b[:, 4 * j:4 * j + 4, :],
                              in_=wkr[:, 4 * j:4 * j + 4, :])
        nc.gpsimd.dma_start(out=eb_sb[:], in_=ebias[:])
        nc.gpsimd.dma_start(out=tri_sb[:], in_=trimask[:])
        nc.gpsimd.dma_start(out=bq_sb[:], in_=bqc[:])
        nc.gpsimd.dma_start(out=bk_sb[:], in_=bkc[:])
        nc.gpsimd.dma_start(out=bv_sb[:], in_=bvr[:])
        nc.gpsimd.dma_start(out=cos_sb[:], in_=cosT[:])
        nc.gpsimd.dma_start(out=ns_sb[:], in_=nsT[:])
        nc.gpsimd.dma_start(out=bvb_sb[:], in_=bvr[0:1, :].to_broadcast([P, M]))

        def rope_inplace(dst, tmp_src, ts0):
            """dst[0:ROT, 512] fp16 <- rope(tmp_src rows 0:ROT) in place.
            tmp_src rows are pre-rope biased values; dst may alias tmp_src."""
            sh = rpool.tile([ROT, 512], F16, tag="sh")
            nc.sync.dma_start(out=sh[0:HALF], in_=tmp_src[HALF:ROT])
            nc.sync.dma_start(out=sh[HALF:ROT], in_=tmp_src[0:HALF])
            rot = rpool.tile([ROT, 512], F16, tag="rot")
            nc.vector.tensor_tensor(rot[:], sh[:], ns_sb[:, ts0:ts0 + 512],
                                    mybir.AluOpType.mult)
            tcos = rpool.tile([ROT, 512], F16, tag="tcos")
            nc.vector.tensor_tensor(tcos[:], tmp_src[:ROT], cos_sb[:, ts0:ts0 + 512],
                                    mybir.AluOpType.mult)
            nc.vector.tensor_tensor(dst[0:ROT], tcos[:], rot[:],
                                    mybir.AluOpType.add)

        class Ph3:
            """Output projection for chunk ic; matmuls are dispensed one at a
            time (step) so they weave between attention matmuls."""

            def __init__(self, ic, attn, pools=None):
                self.ic = ic
                self.attn = attn
                self.items = [(co, mt) for co in range(CT) for mt in range(H_LOC)]
                self.pos = 0
                self.ps = None
                self.pools = pools or [psA]

            def step(self, n=1):
                for _ in range(n):
                    if self.pos >= len(self.items):
                        return
                    co, mt = self.items[self.pos]
                    self.pos += 1
                    if mt == 0:
                        pool = self.pools[co % len(self.pools)]
                        self.ps = pool.tile([P, 512], F32,
                                            tag="psA" if pool is psA else "psS")
                    nc.tensor.matmul(
                        self.ps[:],
                        lhsT=wo_b[:, mt, co * P:(co + 1) * P],
                        rhs=self.attn[:, mt, :],
                        start=(mt == 0), stop=(mt == H_LOC - 1))
                    if mt == H_LOC - 1:
                        if co % 4 == 0:
                            self.ot = oepool.tile([P, 4, 512], F16, tag="ot")
                        nc.vector.tensor_copy(out=self.ot[:, co % 4, :],
                                              in_=self.ps[:])
                        last = self.ic == NT - 1
                        step = 2 if last else 4
                        if co % step == step - 1:
                            j0 = co % 4 - (step - 1)
                            nc.sync.dma_start(
                                out=otr[:, co - step + 1:co + 1,
                                        self.ic * 512:self.ic * 512 + 512],
                                in_=self.ot[:, j0:j0 + step, :])

            def finish(self):
                self.step(len(self.items) - self.pos)

        pending = None

        for ic in range(NT):
            ts0 = ic * 512
            x_cur = x0_sl if ic == 0 else x_next

            # ---- proj q ----
            qcur = qpool.tile([P, H_LOC, 512], F16, tag="qcur")
            if ic == 0:
                # ct-major with 4 concurrent PSUM groups: PE tracks the x0/wq
                # DMA chunk arrivals instead of stalling on the first group
                ps_q = [psA.tile([P, 512], F32, tag="psA", name=f"psq{m}")
                        for m in range(3)]
                ps_q.append(psV.tile([P, 512], F32, tag="psV", name="psq3"))
                for ct in range(CT):
                    for mt in range(H_LOC):
                        nc.tensor.matmul(
                            ps_q[mt][:],
                            lhsT=wq_b[:, ct, mt * P:(mt + 1) * P],
                            rhs=x_cur[ct][:],
                            start=(ct == 0), stop=(ct == CT - 1))
                for mt in range(H_LOC):
                    nc.scalar.activation(
                        qcur[:, mt, :], ps_q[mt][:],
                        mybir.ActivationFunctionType.Identity,
                        bias=bq_sb[:, mt:mt + 1], scale=1.0)
                    rope_inplace(qcur[:, mt, :], qcur[:, mt, :], ts0)
            else:
                for mt in range(H_LOC):
                    ps = psA.tile([P, 512], F32, tag="psA")
                    for ct in range(CT):
                        nc.tensor.matmul(
                            ps[:],
                            lhsT=wq_b[:, ct, mt * P:(mt + 1) * P],
                            rhs=x_cur[ct][:],
                            start=(ct == 0), stop=(ct == CT - 1))
                    nc.scalar.activation(
                        qcur[:, mt, :], ps[:],
                        mybir.ActivationFunctionType.Identity,
                        bias=bq_sb[:, mt:mt + 1], scale=1.0)
                    rope_inplace(qcur[:, mt, :], qcur[:, mt, :], ts0)

            if ic == 0:
                for j in range(H_LOC):
                    nc.gpsimd.dma_start(out=wo_b[:, j:j + 1, :],
                                        in_=wor[:, j:j + 1, :])

            def proj_k():
                for mt in range(H_LOC):
                    ps = psA.tile([P, 512], F32, tag="psA")
                    for ct in range(CT):
                        nc.tensor.matmul(
                            ps[:],
                            lhsT=wk_b[:, ct, mt * P:(mt + 1) * P],
                            rhs=x_cur[ct][:],
                            start=(ct == 0), stop=(ct == CT - 1))
                    nc.scalar.activation(
                        k_res[ROT:P, mt, ts0:ts0 + 512], ps[ROT:P],
                        mybir.ActivationFunctionType.Identity,
                        bias=bk_sb[ROT:P, mt:mt + 1], scale=1.0)
                    ktmp = rpool.tile([ROT, 512], F16, tag="ktmp")
                    nc.scalar.activation(
                        ktmp[:], ps[0:ROT],
                        mybir.ActivationFunctionType.Identity,
                        bias=bk_sb[0:ROT, mt:mt + 1], scale=1.0)
                    rope_inplace(k_res[:, mt, ts0:ts0 + 512], ktmp[:], ts0)

            def proj_v():
                for tt in range(4):
                    ps = psA.tile([P, M], F32, tag="psA")
                    for ct in range(CT):
                        nc.tensor.matmul(
                            ps[:],
                            lhsT=x_cur[ct][:, tt * P:(tt + 1) * P],
                            rhs=wv_b[:, ct, :],
                            start=(ct == 0), stop=(ct == CT - 1))
                    nc.vector.tensor_tensor(
                        v_res[:, 4 * ic + tt, :], ps[:], bvb_sb[:],
                        mybir.AluOpType.add)

            if ic == 0:
                # wk lands last on the SP queue: fill the gap with proj v
                proj_v()
                proj_k()
            else:
                proj_k()
                proj_v()

            if ic + 1 < NT:
                x_next = load_x(ic + 1)

            # ---- attention for chunk ic (weaving ph3 of chunk ic-1) ----
            attn = atpool.tile([P, H_LOC, 512], F16, tag="attn")
            njt = 4 * ic + 4
            slots_left = H_LOC * njt
            for h in range(H_LOC):
                ps_av = psV.tile([P, 512], F32, tag="psV")
                exsum = espool.tile([P, 512], F16, tag="exsum")
                prev = None  # (ex tile, c0) awaiting its av matmul
                for jt in range(njt):
                    d = jt - 4 * ic
                    c0 = 128 * d if d > 0 else 0
                    ps_s = psS.tile([P, 512], F32, tag="psS")
                    nc.tensor.matmul(
                        ps_s[:, c0:],
                        lhsT=k_res[:, h, jt * P:(jt + 1) * P],
                        rhs=qcur[:, h, c0:],
                        start=True, stop=True)
                    ex = expool.tile([P, 512], F16, tag="ex")
                    nc.scalar.activation(
                        ex[:, c0:], ps_s[:, c0:],
                        mybir.ActivationFunctionType.Exp,
                        bias=eb_sb[:, 0:1], scale=SCALE)
                    if d >= 0:
                        # causal triangle: for every diagonal tile the global
                        # query base ts0+c0 equals the key base jt*P, so one
                        # [P,P] keep-where-col>=row mask serves them all
                        nc.vector.tensor_tensor(
                            ex[:, c0:c0 + P], ex[:, c0:c0 + P], tri_sb[:],
                            mybir.AluOpType.mult)
                    if pending is not None:
                        pending.step(1)
                    slots_left -= 1
                    if prev is not None:
                        pex, pc0, pjt = prev
                        nc.tensor.matmul(
                            ps_av[:, pc0:],
                            lhsT=v_res[:, pjt, h * HS:(h + 1) * HS],
                            rhs=pex[:, pc0:],
                            start=(pjt == 0), stop=False,
                            skip_group_check=True)
                    with nc.allow_low_precision(reason="fp16 softmax denom"):
                        if jt == 0:
                            if ic == 0:
                                nc.vector.tensor_copy(out=exsum[:], in_=ex[:])
                        elif jt == 1 and ic > 0:
                            nc.vector.tensor_tensor(
                                exsum[:], prev[0][:], ex[:],
                                mybir.AluOpType.add)
                        else:
                            nc.vector.tensor_tensor(
                                exsum[:, c0:], exsum[:, c0:], ex[:, c0:],
                                mybir.AluOpType.add)
                    prev = (ex, c0, jt)
                pex, pc0, pjt = prev
                nc.tensor.matmul(
                    ps_av[:, pc0:],
                    lhsT=v_res[:, pjt, h * HS:(h + 1) * HS],
                    rhs=pex[:, pc0:],
                    start=(pjt == 0), stop=True,
                    skip_group_check=True)
                rden = rdpool.tile([P, 512], F16, tag="rden")
                nc.gpsimd.partition_all_reduce(
                    rden[:], exsum[:], channels=P, reduce_op=bass_isa.ReduceOp.add)
                with nc.allow_low_precision(reason="softmax reciprocal"):
                    nc.vector.reciprocal(rden[:], rden[:])
                    nc.vector.tensor_tensor(
                        attn[:, h, :], ps_av[:], rden[:],
                        mybir.AluOpType.mult)

            if pending is not None:
                pending.finish()
            pending = Ph3(ic, attn,
                          pools=[psA, psS] if ic == NT - 1 else None)

        pending.finish()

    nc.finalize()
    return nc


def get_nc(phases=(1, 2, 3)):
    if phases not in _NC_CACHE:
        _NC_CACHE[phases] = _build(phases)
    return _NC_CACHE[phases]


def _rope_tables():
    inv_freq = 1.0 / (BASE ** (np.arange(0, ROT, 2, dtype=np.float64) / ROT))
    freqs = np.arange(T, dtype=np.float64)[:, None] * inv_freq[None, :]  # [T, 32]
    cos_h = np.cos(freqs).T.astype(np.float32)   # [32, T]
    sin_h = np.sin(freqs).T.astype(np.float32)
    cosT = np.concatenate([cos_h, cos_h], axis=0)          # [64, T]
    nsT = np.concatenate([-sin_h, sin_h], axis=0)          # [64, T] signed sin
    return (np.ascontiguousarray(cosT.astype(np.float16)),
            np.ascontiguousarray(nsT.astype(np.float16)))


def make_in_maps(x, Wq, bq, Wk, bk, Wv, bv, Wo, bo):
    cosT, nsT = _rope_tables()
    in_maps = []
    for c in range(N_CORES):
        b, g = divmod(c, TPG)
        ms = slice(g * M, (g + 1) * M)
        in_maps.append({
            "xh": np.ascontiguousarray(x[b].T.astype(np.float16)),
            "wqT": np.ascontiguousarray(Wq[ms].T.astype(np.float16)),
            "wkT": np.ascontiguousarray(Wk[ms].T.astype(np.float16)),
            "wvT": np.ascontiguousarray(Wv[ms].T.astype(np.float16)),
            "woT": np.ascontiguousarray(Wo[:, ms].T.astype(np.float16)),
            "bqc": np.ascontiguousarray(bq[ms].reshape(H_LOC, P).T),
            "bkc": np.ascontiguousarray(bk[ms].reshape(H_LOC, P).T),
            "bvr": np.ascontiguousarray(bv[ms].reshape(1, M)),
            "ebias": np.full((P, 1), EXP_BIAS, np.float32),
            "trimask": np.triu(np.ones((P, P), np.float16)),
            "cosT": cosT,
            "nsT": nsT,
        })
    return in_maps


def assemble(results, bo):
    out = np.empty((B, T, C), dtype=np.float32)
    for b in range(B):
        acc = results[b * TPG]["outT"].astype(np.float32)
        for g in range(1, TPG):
            acc = acc + results[b * TPG + g]["outT"].astype(np.float32)
        out[b] = acc.T + bo[None, :]
    return out


def kernel(x, Wq, bq, Wk, bk, Wv, bv, Wo, bo):
    nc = get_nc()
    in_maps = make_in_maps(np.asarray(x, np.float32),
                           np.asarray(Wq, np.float32), np.asarray(bq, np.float32),
                           np.asarray(Wk, np.float32), np.asarray(bk, np.float32),
                           np.asarray(Wv, np.float32), np.asarray(bv, np.float32),
                           np.asarray(Wo, np.float32), np.asarray(bo, np.float32))
    res = run_bass_kernel_spmd(nc, in_maps, list(range(N_CORES)))
    return assemble(res.results, np.asarray(bo, np.float32))


# revision 13
# speedup vs baseline: 1.0118x; 1.0118x over previous
"""Trainium2 Bass kernel for a full causal MHA layer (B=2, T=2048, C=2048, H=16,
partial RoPE on first 64 dims of each 128-dim head).

Sharding over 8 cores: core c handles batch b=c//4 and heads [4g, 4g+4), g=c%4
(tensor-parallel over heads x data-parallel over batch).

Fully fused single pass per core, fp16 data plane (fp32 PSUM accumulation):
  for each 512-token chunk ic:
    proj q/k (fp16 weights stationary, fp16 x moving), bias+partial-RoPE,
      q and k stay resident in SBUF (no DRAM spills)
    proj v -> v_res [key, jt, m] fp16 resident
    attention for chunk ic over heads h: per key-tile jt
      scoresT[k,q] (k_res stationary fp16, q moving fp16)
      -> exp(scale*s - 10*ln2) -> ex fp16 (Act), triangle mask on diagonal
         tiles only (gpsimd affine_select), exact causal col-trimming
      -> av accumulation outT[d,q] via PE; softmax denominator via DVE
         exsum adds + gpsimd partition_all_reduce (no PE ones-matmuls)
      output-projection matmuls of chunk ic-1 are woven between attention
      matmuls to keep PE busy during Act-latency windows
    phase3(ic): out partial outT[c,q] = sum_mt woT attn, DVE evict fp16, DMA
Host: slices inputs per core (fp16), sums the 4 TP partials per batch + bo.
"""

import math

import numpy as np

import concourse.bass as bass
import concourse.bass_isa as bass_isa
import concourse.mybir as mybir
import concourse.tile as tile
from concourse import bacc
from concourse.bass_utils import run_bass_kernel_spmd

F32 = mybir.dt.float32
F16 = mybir.dt.float16

B, T, C = 2, 2048, 2048
H = 16
HS = 128
ROT = 64
HALF = 32
BASE = 10000.0

N_CORES = 8
TPG = 4                # TP group size (heads split)
H_LOC = H // TPG       # 4 heads per core
M = H_LOC * HS         # 512 local head-dim columns
SCALE = 1.0 / math.sqrt(HS)
EXP_BIAS = -10.0 * math.log(2.0)   # exp(s*SCALE - 10ln2): keeps fp16 in range

P = 128
NT = T // 512          # 4 t-chunks of 512
CT = C // P            # 16 contraction tiles
JT = T // P            # 16 key tiles per head

_NC_CACHE = {}


def _build(phases=(1, 2, 3)):
    nc = bacc.Bacc(None, target_bir_lowering=False)

    xh = nc.declare_dram_parameter("xh", [C, T], F16, isOutput=False)
    wqT = nc.declare_dram_parameter("wqT", [C, M], F16, isOutput=False)
    wkT = nc.declare_dram_parameter("wkT", [C, M], F16, isOutput=False)
    wvT = nc.declare_dram_parameter("wvT", [C, M], F16, isOutput=False)
    woT = nc.declare_dram_parameter("woT", [M, C], F16, isOutput=False)
    bqc = nc.declare_dram_parameter("bqc", [P, H_LOC], F32, isOutput=False)
    bkc = nc.declare_dram_parameter("bkc", [P, H_LOC], F32, isOutput=False)
    bvr = nc.declare_dram_parameter("bvr", [1, M], F32, isOutput=False)
    ebias = nc.declare_dram_parameter("ebias", [P, 1], F32, isOutput=False)
    trimask = nc.declare_dram_parameter("trimask", [P, P], F16, isOutput=False)
    cosT = nc.declare_dram_parameter("cosT", [ROT, T], F16, isOutput=False)
    nsT = nc.declare_dram_parameter("nsT", [ROT, T], F16, isOutput=False)
    outT = nc.declare_dram_parameter("outT", [C, T], F16, isOutput=True)

    with tile.TileContext(nc) as tc, \
         tc.tile_pool(name="const", bufs=1) as const, \
         tc.tile_pool(name="wp", bufs=CT) as wpool, \
         tc.tile_pool(name="wop", bufs=H_LOC) as wopool, \
         tc.tile_pool(name="xp", bufs=CT) as xpool, \
         tc.tile_pool(name="qc", bufs=2) as qpool, \
         tc.tile_pool(name="at", bufs=2) as atpool, \
         tc.tile_pool(name="rp", bufs=6) as rpool, \
         tc.tile_pool(name="exp", bufs=6) as expool, \
         tc.tile_pool(name="exs", bufs=2) as espool, \
         tc.tile_pool(name="rd", bufs=2) as rdpool, \
         tc.tile_pool(name="oe", bufs=3) as oepool, \
         tc.tile_pool(name="psA", bufs=3, space="PSUM") as psA, \
         tc.tile_pool(name="psS", bufs=3, space="PSUM") as psS, \
         tc.tile_pool(name="psV", bufs=2, space="PSUM") as psV:

        cos_sb = const.tile([ROT, T], F16, tag="cos")
        ns_sb = const.tile([ROT, T], F16, tag="ns")
        bq_sb = const.tile([P, H_LOC], F32, tag="bq")
        bk_sb = const.tile([P, H_LOC], F32, tag="bk")
        bv_sb = const.tile([1, M], F32, tag="bv")
        bvb_sb = const.tile([P, M], F32, tag="bvb")
        eb_sb = const.tile([P, 1], F32, tag="ebias")
        tri_sb = const.tile([P, P], F16, tag="trimask")
        k_res = const.tile([P, H_LOC, T], F16, tag="kres")
        v_res = const.tile([P, JT, M], F16, tag="vres")
        wq_b = const.tile([P, CT, M], F16, tag="wqb")
        wk_b = const.tile([P, CT, M], F16, tag="wkb")
        wv_b = const.tile([P, CT, M], F16, tag="wvb")
        wo_b = const.tile([P, H_LOC, C], F16, tag="wob")
        wqr = wqT[:].rearrange("(ct p) m -> p ct m", p=P)
        wkr = wkT[:].rearrange("(ct p) m -> p ct m", p=P)
        wvr = wvT[:].rearrange("(ct p) m -> p ct m", p=P)
        wor = woT[:].rearrange("(mt p) c -> p mt c", p=P)

        xr = xh[:].rearrange("(ct p) t -> p ct t", p=P)
        otr = outT[:].rearrange("(co p) t -> p co t", p=P)

        def load_x(ic):
            ts0 = ic * 512
            xb = xpool.tile([P, CT, 512], F16, tag="xb", name=f"xb{ic}", bufs=2)
            for j in range(4):
                nc.sync.dma_start(out=xb[:, 4 * j:4 * j + 4, :],
                                  in_=xr[:, 4 * j:4 * j + 4, ts0:ts0 + 512])
            return [xb[:, ct, :] for ct in range(CT)]

        # startup: x0 streams on the SP queue while wq streams on the Act
        # queue; chunked so HWDGE fixed overheads don't pace the start.
        xb0 = xpool.tile([P, CT, 512], F16, tag="xb", name="xb0", bufs=2)
        x0_chunks = [(2 * j, 2 * j + 2) for j in range(8)]
        for a, b in x0_chunks:
            nc.sync.dma_start(out=xb0[:, a:b, :], in_=xr[:, a:b, 0:512])
        x0_sl = [xb0[:, ct, :] for ct in range(CT)]
        for a, b in x0_chunks:
            nc.scalar.dma_start(out=wq_b[:, a:b, :], in_=wqr[:, a:b, :])
        for j in range(4):
            nc.sync.dma_start(out=wv_b[:, 4 * j:4 * j + 4, :],
                              in_=wvr[:, 4 * j:4 * j + 4, :])
        for j in range(4):
            nc.sync.dma_start(out=wk_b[:, 4 * j:4 * j + 4, :],
                              in_=wkr[:, 4 * j:4 * j + 4, :])
        nc.gpsimd.dma_start(out=eb_sb[:], in_=ebias[:])
        nc.gpsimd.dma_start(out=tri_sb[:], in_=trimask[:])
        nc.gpsimd.dma_start(out=bq_sb[:], in_=bqc[:])
        nc.gpsimd.dma_start(out=bk_sb[:], in_=bkc[:])
        nc.gpsimd.dma_start(out=bv_sb[:], in_=bvr[:])
        nc.gpsimd.dma_start(out=cos_sb[:], in_=cosT[:])
        nc.gpsimd.dma_start(out=ns_sb[:], in_=nsT[:])
        nc.gpsimd.dma_start(out=bvb_sb[:], in_=bvr[0:1, :].to_broadcast([P, M]))

        def rope_inplace(dst, tmp_src, ts0):
            """dst[0:ROT, 512] fp16 <- rope(tmp_src rows 0:ROT) in place.
            tmp_src rows are pre-rope biased values; dst may alias tmp_src."""
            sh = rpool.tile([ROT, 512], F16, tag="sh")
            nc.sync.dma_start(out=sh[0:HALF], in_=tmp_src[HALF:ROT])
            nc.sync.dma_start(out=sh[HALF:ROT], in_=tmp_src[0:HALF])
            rot = rpool.tile([ROT, 512], F16, tag="rot")
            nc.vector.tensor_tensor(rot[:], sh[:], ns_sb[:, ts0:ts0 + 512],
                                    mybir.AluOpType.mult)
            tcos = rpool.tile([ROT, 512], F16, tag="tcos")
            nc.vector.tensor_tensor(tcos[:], tmp_src[:ROT], cos_sb[:, ts0:ts0 + 512],
                                    mybir.AluOpType.mult)
            nc.vector.tensor_tensor(dst[0:ROT], tcos[:], rot[:],
                                    mybir.AluOpType.add)

        class Ph3:
            """Output projection for chunk ic; matmuls are dispensed one at a
            time (step) so they weave between attention matmuls."""

            def __init__(self, ic, attn, pools=None):
                self.ic = ic
                self.attn = attn
                self.items = [(co, mt) for co in range(CT) for mt in range(H_LOC)]
                self.pos = 0
                self.ps = None
                self.pools = pools or [psA]
                self.finishing = False

            def step(self, n=1):
                for _ in range(n):
                    if self.pos >= len(self.items):
                        return
                    co, mt = self.items[self.pos]
                    self.pos += 1
                    if mt == 0:
                        pool = self.pools[co % len(self.pools)]
                        self.ps = pool.tile([P, 512], F32,
                                            tag="psA" if pool is psA else "psS")
                    nc.tensor.matmul(
                        self.ps[:],
                        lhsT=wo_b[:, mt, co * P:(co + 1) * P],
                        rhs=self.attn[:, mt, :],
                        start=(mt == 0), stop=(mt == H_LOC - 1))
                    if mt == H_LOC - 1:
                        if co % 4 == 0:
                            self.ot = oepool.tile([P, 4, 512], F16, tag="ot")
                        if self.finishing and co % 2 == 1:
                            # post-attention block: DVE is draining attention
                            # tail work, so alternate evicts onto idle Act
                            nc.scalar.copy(self.ot[:, co % 4, :], self.ps[:])
                        else:
                            nc.vector.tensor_copy(out=self.ot[:, co % 4, :],
                                                  in_=self.ps[:])
                        last = self.ic == NT - 1
                        step = 2 if last else 4
                        if co % step == step - 1:
                            j0 = co % 4 - (step - 1)
                            nc.sync.dma_start(
                                out=otr[:, co - step + 1:co + 1,
                                        self.ic * 512:self.ic * 512 + 512],
                                in_=self.ot[:, j0:j0 + step, :])

            def finish(self):
                self.finishing = True
                self.step(len(self.items) - self.pos)

        pending = None

        for ic in range(NT):
            ts0 = ic * 512
            x_cur = x0_sl if ic == 0 else x_next

            # ---- proj q ----
            qcur = qpool.tile([P, H_LOC, 512], F16, tag="qcur")
            if ic == 0:
                # ct-major with 4 concurrent PSUM groups: PE tracks the x0/wq
                # DMA chunk arrivals instead of stalling on the first group
                ps_q = [psA.tile([P, 512], F32, tag="psA", name=f"psq{m}")
                        for m in range(3)]
                ps_q.append(psV.tile([P, 512], F32, tag="psV", name="psq3"))
                for ct in range(CT):
                    for mt in range(H_LOC):
                        nc.tensor.matmul(
                            ps_q[mt][:],
                            lhsT=wq_b[:, ct, mt * P:(mt + 1) * P],
                            rhs=x_cur[ct][:],
                            start=(ct == 0), stop=(ct == CT - 1))
                for mt in range(H_LOC):
                    nc.scalar.activation(
                        qcur[:, mt, :], ps_q[mt][:],
                        mybir.ActivationFunctionType.Identity,
                        bias=bq_sb[:, mt:mt + 1], scale=1.0)
                    rope_inplace(qcur[:, mt, :], qcur[:, mt, :], ts0)
            else:
                for mt in range(H_LOC):
                    ps = psA.tile([P, 512], F32, tag="psA")
                    for ct in range(CT):
                        nc.tensor.matmul(
                            ps[:],
                            lhsT=wq_b[:, ct, mt * P:(mt + 1) * P],
                            rhs=x_cur[ct][:],
                            start=(ct == 0), stop=(ct == CT - 1))
                    nc.scalar.activation(
                        qcur[:, mt, :], ps[:],
                        mybir.ActivationFunctionType.Identity,
                        bias=bq_sb[:, mt:mt + 1], scale=1.0)
                    rope_inplace(qcur[:, mt, :], qcur[:, mt, :], ts0)

            if ic == 0:
                for j in range(H_LOC):
                    nc.gpsimd.dma_start(out=wo_b[:, j:j + 1, :],
                                        in_=wor[:, j:j + 1, :])

            def proj_k():
                for mt in range(H_LOC):
                    ps = psA.tile([P, 512], F32, tag="psA")
                    for ct in range(CT):
                        nc.tensor.matmul(
                            ps[:],
                            lhsT=wk_b[:, ct, mt * P:(mt + 1) * P],
                            rhs=x_cur[ct][:],
                            start=(ct == 0), stop=(ct == CT - 1))
                    nc.scalar.activation(
                        k_res[ROT:P, mt, ts0:ts0 + 512], ps[ROT:P],
                        mybir.ActivationFunctionType.Identity,
                        bias=bk_sb[ROT:P, mt:mt + 1], scale=1.0)
                    ktmp = rpool.tile([ROT, 512], F16, tag="ktmp")
                    nc.scalar.activation(
                        ktmp[:], ps[0:ROT],
                        mybir.ActivationFunctionType.Identity,
                        bias=bk_sb[0:ROT, mt:mt + 1], scale=1.0)
                    rope_inplace(k_res[:, mt, ts0:ts0 + 512], ktmp[:], ts0)

            def proj_v():
                if ic == 0:
                    # ct-major: track the wv chunk arrivals during startup
                    ps_v = [psA.tile([P, M], F32, tag="psA", name=f"psv{t}")
                            for t in range(3)]
                    ps_v.append(psV.tile([P, M], F32, tag="psV", name="psv3"))
                    for ct in range(CT):
                        for tt in range(4):
                            nc.tensor.matmul(
                                ps_v[tt][:],
                                lhsT=x_cur[ct][:, tt * P:(tt + 1) * P],
                                rhs=wv_b[:, ct, :],
                                start=(ct == 0), stop=(ct == CT - 1))
                    for tt in range(4):
                        nc.vector.tensor_tensor(
                            v_res[:, 4 * ic + tt, :], ps_v[tt][:], bvb_sb[:],
                            mybir.AluOpType.add)
                    return
                for tt in range(4):
                    ps = psA.tile([P, M], F32, tag="psA")
                    for ct in range(CT):
                        nc.tensor.matmul(
                            ps[:],
                            lhsT=x_cur[ct][:, tt * P:(tt + 1) * P],
                            rhs=wv_b[:, ct, :],
                            start=(ct == 0), stop=(ct == CT - 1))
                    nc.vector.tensor_tensor(
                        v_res[:, 4 * ic + tt, :], ps[:], bvb_sb[:],
                        mybir.AluOpType.add)

            if ic == 0:
                # wk lands last on the SP queue: fill the gap with proj v
                proj_v()
                proj_k()
            else:
                proj_k()
                proj_v()

            if ic + 1 < NT:
                x_next = load_x(ic + 1)

            # ---- attention for chunk ic (weaving ph3 of chunk ic-1) ----
            attn = atpool.tile([P, H_LOC, 512], F16, tag="attn")
            njt = 4 * ic + 4
            slots_left = H_LOC * njt
            for h in range(H_LOC):
                ps_av = psV.tile([P, 512], F32, tag="psV")
                exsum = espool.tile([P, 512], F16, tag="exsum")
                prev = None  # (ex tile, c0) awaiting its av matmul
                for jt in range(njt):
                    d = jt - 4 * ic
                    c0 = 128 * d if d > 0 else 0
                    ps_s = psS.tile([P, 512], F32, tag="psS")
                    nc.tensor.matmul(
                        ps_s[:, c0:],
                        lhsT=k_res[:, h, jt * P:(jt + 1) * P],
                        rhs=qcur[:, h, c0:],
                        start=True, stop=True)
                    ex = expool.tile([P, 512], F16, tag="ex")
                    nc.scalar.activation(
                        ex[:, c0:], ps_s[:, c0:],
                        mybir.ActivationFunctionType.Exp,
                        bias=eb_sb[:, 0:1], scale=SCALE)
                    if d >= 0:
                        # causal triangle: for every diagonal tile the global
                        # query base ts0+c0 equals the key base jt*P, so one
                        # [P,P] keep-where-col>=row mask serves them all
                        nc.vector.tensor_tensor(
                            ex[:, c0:c0 + P], ex[:, c0:c0 + P], tri_sb[:],
                            mybir.AluOpType.mult)
                    if pending is not None:
                        pending.step(1)
                    slots_left -= 1
                    if prev is not None:
                        pex, pc0, pjt = prev
                        nc.tensor.matmul(
                            ps_av[:, pc0:],
                            lhsT=v_res[:, pjt, h * HS:(h + 1) * HS],
                            rhs=pex[:, pc0:],
                            start=(pjt == 0), stop=False,
                            skip_group_check=True)
                    with nc.allow_low_precision(reason="fp16 softmax denom"):
                        if jt == 0:
                            if ic == 0:
                                nc.vector.tensor_copy(out=exsum[:], in_=ex[:])
                        elif jt == 1 and ic > 0:
                            nc.vector.tensor_tensor(
                                exsum[:], prev[0][:], ex[:],
                                mybir.AluOpType.add)
                        else:
                            nc.vector.tensor_tensor(
                                exsum[:, c0:], exsum[:, c0:], ex[:, c0:],
                                mybir.AluOpType.add)
                    prev = (ex, c0, jt)
                pex, pc0, pjt = prev
                nc.tensor.matmul(
                    ps_av[:, pc0:],
                    lhsT=v_res[:, pjt, h * HS:(h + 1) * HS],
                    rhs=pex[:, pc0:],
                    start=(pjt == 0), stop=True,
                    skip_group_check=True)
                rden = rdpool.tile([P, 512], F16, tag="rden")
                nc.gpsimd.partition_all_reduce(
                    rden[:], exsum[:], channels=P, reduce_op=bass_isa.ReduceOp.add)
                with nc.allow_low_precision(reason="softmax reciprocal"):
                    nc.vector.reciprocal(rden[:], rden[:])
                    nc.vector.tensor_tensor(
                        attn[:, h, :], ps_av[:], rden[:],
                        mybir.AluOpType.mult)

            if pending is not None:
                pending.finish()
            pending = Ph3(ic, attn,
                          pools=[psA, psS] if ic == NT - 1 else None)

        pending.finish()

    nc.finalize()
    return nc


def get_nc(phases=(1, 2, 3)):
    if phases not in _NC_CACHE:
        _NC_CACHE[phases] = _build(phases)
    return _NC_CACHE[phases]


def _rope_tables():
    inv_freq = 1.0 / (BASE ** (np.arange(0, ROT, 2, dtype=np.float64) / ROT))
    freqs = np.arange(T, dtype=np.float64)[:, None] * inv_freq[None, :]  # [T, 32]
    cos_h = np.cos(freqs).T.astype(np.float32)   # [32, T]
    sin_h = np.sin(freqs).T.astype(np.float32)
    cosT = np.concatenate([cos_h, cos_h], axis=0)          # [64, T]
    nsT = np.concatenate([-sin_h, sin_h], axis=0)          # [64, T] signed sin
    return (np.ascontiguousarray(cosT.astype(np.float16)),
            np.ascontiguousarray(nsT.astype(np.float16)))


def make_in_maps(x, Wq, bq, Wk, bk, Wv, bv, Wo, bo):
    cosT, nsT = _rope_tables()
    in_maps = []
    for c in range(N_CORES):
        b, g = divmod(c, TPG)
        ms = slice(g * M, (g + 1) * M)
        in_maps.append({
            "xh": np.ascontiguousarray(x[b].T.astype(np.float16)),
            "wqT": np.ascontiguousarray(Wq[ms].T.astype(np.float16)),
            "wkT": np.ascontiguousarray(Wk[ms].T.astype(np.float16)),
            "wvT": np.ascontiguousarray(Wv[ms].T.astype(np.float16)),
            "woT": np.ascontiguousarray(Wo[:, ms].T.astype(np.float16)),
            "bqc": np.ascontiguousarray(bq[ms].reshape(H_LOC, P).T),
            "bkc": np.ascontiguousarray(bk[ms].reshape(H_LOC, P).T),
            "bvr": np.ascontiguousarray(bv[ms].reshape(1, M)),
            "ebias": np.full((P, 1), EXP_BIAS, np.float32),
            "trimask": np.triu(np.ones((P, P), np.float16)),
            "cosT": cosT,
            "nsT": nsT,
        })
    return in_maps


def assemble(results, bo):
    out = np.empty((B, T, C), dtype=np.float32)
    for b in range(B):
        acc = results[b * TPG]["outT"].astype(np.float32)
        for g in range(1, TPG):
            acc = acc + results[b * TPG + g]["outT"].astype(np.float32)
        out[b] = acc.T + bo[None, :]
    return out


def kernel(x, Wq, bq, Wk, bk, Wv, bv, Wo, bo):
    nc = get_nc()
    in_maps = make_in_maps(np.asarray(x, np.float32),
                           np.asarray(Wq, np.float32), np.asarray(bq, np.float32),
                           np.asarray(Wk, np.float32), np.asarray(bk, np.float32),
                           np.asarray(Wv, np.float32), np.asarray(bv, np.float32),
                           np.asarray(Wo, np.float32), np.asarray(bo, np.float32))
    res = run_bass_kernel_spmd(nc, in_maps, list(range(N_CORES)))
    return assemble(res.results, np.asarray(bo, np.float32))


# revision 19
# speedup vs baseline: 1.1487x; 1.1353x over previous
"""Trainium2 Bass kernel for a full causal MHA layer (B=2, T=2048, C=2048, H=16,
partial RoPE on first 64 dims of each 128-dim head).

Sharding over 8 cores: core c handles batch b=c//4 and heads [4g, 4g+4), g=c%4
(tensor-parallel over heads x data-parallel over batch).

Fully fused single pass per core, fp16 data plane (fp32 PSUM accumulation):
  for each 512-token chunk ic:
    proj q/k (fp16 weights stationary, fp16 x moving), bias+partial-RoPE,
      q and k stay resident in SBUF (no DRAM spills)
    proj v -> v_res [key, jt, m] fp16 resident
    attention for chunk ic over heads h: per key-tile jt
      scoresT[k,q] (k_res stationary fp16, q moving fp16)
      -> exp(scale*s - 10*ln2) -> ex fp16 (Act), triangle mask on diagonal
         tiles only (DVE mult by a const [128,128] triu mask), exact causal
         col-trimming
      -> av accumulation outT[d,q] via PE; softmax denominator via DVE
         exsum adds + gpsimd partition_all_reduce (no PE ones-matmuls)
      output-projection matmuls of chunk ic-1 are woven between attention
      matmuls to keep PE busy during Act-latency windows
    phase3(ic): out partial outT[c,q] = sum_mt woT attn, DVE evict fp16, DMA
Host: slices inputs per core (fp16), sums the 4 TP partials per batch + bo.
"""

import math

import numpy as np

import concourse.bass_isa as bass_isa
import concourse.mybir as mybir
import concourse.tile as tile
from concourse import bacc
from concourse.bass_utils import run_bass_kernel_spmd

F32 = mybir.dt.float32
F16 = mybir.dt.float16

B, T, C = 2, 2048, 2048
H = 16
HS = 128
ROT = 64
HALF = 32
BASE = 10000.0

N_CORES = 8
TPG = 4                # TP group size (heads split)
H_LOC = H // TPG       # 4 heads per core
M = H_LOC * HS         # 512 local head-dim columns
SCALE = 1.0 / math.sqrt(HS)
EXP_BIAS = -10.0 * math.log(2.0)   # exp(s*SCALE - 10ln2): keeps fp16 in range

P = 128
NT = T // 512          # 4 t-chunks of 512
CT = C // P            # 16 contraction tiles
JT = T // P            # 16 key tiles per head

_NC_CACHE = {}


def _build(phases=(1, 2, 3)):
    nc = bacc.Bacc(None, target_bir_lowering=False)

    xh = nc.declare_dram_parameter("xh", [C, T], F16, isOutput=False)
    wqT = nc.declare_dram_parameter("wqT", [C, M], F16, isOutput=False)
    wkT = nc.declare_dram_parameter("wkT", [C, M], F16, isOutput=False)
    wvT = nc.declare_dram_parameter("wvT", [C, M], F16, isOutput=False)
    woT = nc.declare_dram_parameter("woT", [M, C], F16, isOutput=False)
    bqc = nc.declare_dram_parameter("bqc", [P, H_LOC], F32, isOutput=False)
    bkc = nc.declare_dram_parameter("bkc", [P, H_LOC], F32, isOutput=False)
    bvr = nc.declare_dram_parameter("bvr", [1, M], F32, isOutput=False)
    ebias = nc.declare_dram_parameter("ebias", [P, 1], F32, isOutput=False)
    trimask = nc.declare_dram_parameter("trimask", [P, P], F16, isOutput=False)
    cosT = nc.declare_dram_parameter("cosT", [ROT, T], F16, isOutput=False)
    nsT = nc.declare_dram_parameter("nsT", [ROT, T], F16, isOutput=False)
    outT = nc.declare_dram_parameter("outT", [C, T], F16, isOutput=True)

    with tile.TileContext(nc) as tc, \
         tc.tile_pool(name="const", bufs=1) as const, \
         tc.tile_pool(name="xp", bufs=CT) as xpool, \
         tc.tile_pool(name="qc", bufs=2) as qpool, \
         tc.tile_pool(name="at", bufs=2) as atpool, \
         tc.tile_pool(name="rp", bufs=6) as rpool, \
         tc.tile_pool(name="exp", bufs=6) as expool, \
         tc.tile_pool(name="exs", bufs=2) as espool, \
         tc.tile_pool(name="rd", bufs=2) as rdpool, \
         tc.tile_pool(name="oe", bufs=3) as oepool, \
         tc.tile_pool(name="psA", bufs=3, space="PSUM") as psA, \
         tc.tile_pool(name="psS", bufs=3, space="PSUM") as psS, \
         tc.tile_pool(name="psV", bufs=2, space="PSUM") as psV:

        cos_sb = const.tile([ROT, T], F16, tag="cos")
        ns_sb = const.tile([ROT, T], F16, tag="ns")
        bq_sb = const.tile([P, H_LOC], F32, tag="bq")
        bk_sb = const.tile([P, H_LOC], F32, tag="bk")
        bv_sb = const.tile([1, M], F32, tag="bv")
        bvb_sb = const.tile([P, M], F32, tag="bvb")
        eb_sb = const.tile([P, 1], F32, tag="ebias")
        tri_sb = const.tile([P, P], F16, tag="trimask")
        k_res = const.tile([P, H_LOC, T], F16, tag="kres")
        v_res = const.tile([P, JT, M], F16, tag="vres")
        wq_b = const.tile([P, CT, M], F16, tag="wqb")
        wk_b = const.tile([P, CT, M], F16, tag="wkb")
        wv_b = const.tile([P, CT, M], F16, tag="wvb")
        wo_b = const.tile([P, H_LOC, C], F16, tag="wob")
        wqr = wqT[:].rearrange("(ct p) m -> p ct m", p=P)
        wkr = wkT[:].rearrange("(ct p) m -> p ct m", p=P)
        wvr = wvT[:].rearrange("(ct p) m -> p ct m", p=P)
        wor = woT[:].rearrange("(mt p) c -> p mt c", p=P)

        xr = xh[:].rearrange("(ct p) t -> p ct t", p=P)
        otr = outT[:].rearrange("(co p) t -> p co t", p=P)

        def load_x(ic):
            ts0 = ic * 512
            xb = xpool.tile([P, CT, 512], F16, tag="xb", name=f"xb{ic}", bufs=2)
            for j in range(4):
                nc.sync.dma_start(out=xb[:, 4 * j:4 * j + 4, :],
                                  in_=xr[:, 4 * j:4 * j + 4, ts0:ts0 + 512])
            return [xb[:, ct, :] for ct in range(CT)]

        # startup: x0 streams on the SP queue while wq streams on the Act
        # queue; chunked so HWDGE fixed overheads don't pace the start.
        xb0 = xpool.tile([P, CT, 512], F16, tag="xb", name="xb0", bufs=2)
        x0_chunks = [(2 * j, 2 * j + 2) for j in range(8)]
        for a, b in x0_chunks:
            nc.sync.dma_start(out=xb0[:, a:b, :], in_=xr[:, a:b, 0:512])
        x0_sl = [xb0[:, ct, :] for ct in range(CT)]
        for a, b in x0_chunks:
            nc.scalar.dma_start(out=wq_b[:, a:b, :], in_=wqr[:, a:b, :])
        for j in range(4):
            nc.sync.dma_start(out=wv_b[:, 4 * j:4 * j + 4, :],
                              in_=wvr[:, 4 * j:4 * j + 4, :])
        for j in range(4):
            nc.sync.dma_start(out=wk_b[:, 4 * j:4 * j + 4, :],
                              in_=wkr[:, 4 * j:4 * j + 4, :])
        nc.gpsimd.dma_start(out=eb_sb[:], in_=ebias[:])
        nc.gpsimd.dma_start(out=tri_sb[:], in_=trimask[:])
        nc.gpsimd.dma_start(out=bq_sb[:], in_=bqc[:])
        nc.gpsimd.dma_start(out=bk_sb[:], in_=bkc[:])
        nc.gpsimd.dma_start(out=bv_sb[:], in_=bvr[:])
        nc.gpsimd.dma_start(out=cos_sb[:], in_=cosT[:])
        nc.gpsimd.dma_start(out=ns_sb[:], in_=nsT[:])
        nc.gpsimd.dma_start(out=bvb_sb[:], in_=bvr[0:1, :].to_broadcast([P, M]))

        def rope_inplace(dst, tmp_src, ts0):
            """dst[0:ROT, 512] fp16 <- rope(tmp_src rows 0:ROT) in place.
            tmp_src rows are pre-rope biased values; dst may alias tmp_src."""
            sh = rpool.tile([ROT, 512], F16, tag="sh")
            nc.sync.dma_start(out=sh[0:HALF], in_=tmp_src[HALF:ROT])
            nc.sync.dma_start(out=sh[HALF:ROT], in_=tmp_src[0:HALF])
            rot = rpool.tile([ROT, 512], F16, tag="rot")
            nc.vector.tensor_tensor(rot[:], sh[:], ns_sb[:, ts0:ts0 + 512],
                                    mybir.AluOpType.mult)
            tcos = rpool.tile([ROT, 512], F16, tag="tcos")
            nc.vector.tensor_tensor(tcos[:], tmp_src[:ROT], cos_sb[:, ts0:ts0 + 512],
                                    mybir.AluOpType.mult)
            nc.vector.tensor_tensor(dst[0:ROT], tcos[:], rot[:],
                                    mybir.AluOpType.add)

        class Ph3:
            """Output projection for chunk ic; matmuls are dispensed one at a
            time (step) so they weave between attention matmuls."""

            def __init__(self, ic, attn, pools=None):
                self.ic = ic
                self.attn = attn
                self.items = [(co, mt) for co in range(CT) for mt in range(H_LOC)]
                self.pos = 0
                self.ps = None
                self.pools = pools or [psA]
                self.finishing = False

            def step(self, n=1):
                for _ in range(n):
                    if self.pos >= len(self.items):
                        return
                    co, mt = self.items[self.pos]
                    self.pos += 1
                    if mt == 0:
                        pool = self.pools[co % len(self.pools)]
                        self.ps = pool.tile([P, 512], F32,
                                            tag="psA" if pool is psA else "psS")
                    nc.tensor.matmul(
                        self.ps[:],
                        lhsT=wo_b[:, mt, co * P:(co + 1) * P],
                        rhs=self.attn[:, mt, :],
                        start=(mt == 0), stop=(mt == H_LOC - 1))
                    if mt == H_LOC - 1:
                        if co % 4 == 0:
                            self.ot = oepool.tile([P, 4, 512], F16, tag="ot")
                        if self.finishing and co % 2 == 1:
                            # post-attention block: DVE is draining attention
                            # tail work, so alternate evicts onto idle Act
                            nc.scalar.copy(self.ot[:, co % 4, :], self.ps[:])
                        else:
                            nc.vector.tensor_copy(out=self.ot[:, co % 4, :],
                                                  in_=self.ps[:])
                        last = self.ic == NT - 1
                        step = 2 if last else 4
                        if co % step == step - 1:
                            j0 = co % 4 - (step - 1)
                            nc.sync.dma_start(
                                out=otr[:, co - step + 1:co + 1,
                                        self.ic * 512:self.ic * 512 + 512],
                                in_=self.ot[:, j0:j0 + step, :])

            def finish(self):
                self.finishing = True
                self.step(len(self.items) - self.pos)

        pending = None

        for ic in range(NT):
            ts0 = ic * 512
            x_cur = x0_sl if ic == 0 else x_next

            # ---- proj q ----
            qcur = qpool.tile([P, H_LOC, 512], F16, tag="qcur")
            if ic == 0:
                # ct-major with 4 concurrent PSUM groups: PE tracks the x0/wq
                # DMA chunk arrivals instead of stalling on the first group
                ps_q = [psA.tile([P, 512], F32, tag="psA", name=f"psq{m}")
                        for m in range(3)]
                ps_q.append(psV.tile([P, 512], F32, tag="psV", name="psq3"))
                for ct in range(CT):
                    for mt in range(H_LOC):
                        nc.tensor.matmul(
                            ps_q[mt][:],
                            lhsT=wq_b[:, ct, mt * P:(mt + 1) * P],
                            rhs=x_cur[ct][:],
                            start=(ct == 0), stop=(ct == CT - 1))
                for mt in range(H_LOC):
                    nc.scalar.activation(
                        qcur[:, mt, :], ps_q[mt][:],
                        mybir.ActivationFunctionType.Identity,
                        bias=bq_sb[:, mt:mt + 1], scale=1.0)
                    rope_inplace(qcur[:, mt, :], qcur[:, mt, :], ts0)
            else:
                for mt in range(H_LOC):
                    ps = psA.tile([P, 512], F32, tag="psA")
                    for ct in range(CT):
                        nc.tensor.matmul(
                            ps[:],
                            lhsT=wq_b[:, ct, mt * P:(mt + 1) * P],
                            rhs=x_cur[ct][:],
                            start=(ct == 0), stop=(ct == CT - 1))
                    nc.scalar.activation(
                        qcur[:, mt, :], ps[:],
                        mybir.ActivationFunctionType.Identity,
                        bias=bq_sb[:, mt:mt + 1], scale=1.0)
                    rope_inplace(qcur[:, mt, :], qcur[:, mt, :], ts0)

            if ic == 0:
                for j in range(H_LOC):
                    nc.gpsimd.dma_start(out=wo_b[:, j:j + 1, :],
                                        in_=wor[:, j:j + 1, :])

            def proj_k():
                for mt in range(H_LOC):
                    ps = psA.tile([P, 512], F32, tag="psA")
                    for ct in range(CT):
                        nc.tensor.matmul(
                            ps[:],
                            lhsT=wk_b[:, ct, mt * P:(mt + 1) * P],
                            rhs=x_cur[ct][:],
                            start=(ct == 0), stop=(ct == CT - 1))
                    nc.scalar.activation(
                        k_res[ROT:P, mt, ts0:ts0 + 512], ps[ROT:P],
                        mybir.ActivationFunctionType.Identity,
                        bias=bk_sb[ROT:P, mt:mt + 1], scale=1.0)
                    ktmp = rpool.tile([ROT, 512], F16, tag="ktmp")
                    nc.scalar.activation(
                        ktmp[:], ps[0:ROT],
                        mybir.ActivationFunctionType.Identity,
                        bias=bk_sb[0:ROT, mt:mt + 1], scale=1.0)
                    rope_inplace(k_res[:, mt, ts0:ts0 + 512], ktmp[:], ts0)

            def proj_v():
                if ic == 0:
                    # ct-major: track the wv chunk arrivals during startup
                    ps_v = [psA.tile([P, M], F32, tag="psA", name=f"psv{t}")
                            for t in range(3)]
                    ps_v.append(psV.tile([P, M], F32, tag="psV", name="psv3"))
                    for ct in range(CT):
                        for tt in range(4):
                            nc.tensor.matmul(
                                ps_v[tt][:],
                                lhsT=x_cur[ct][:, tt * P:(tt + 1) * P],
                                rhs=wv_b[:, ct, :],
                                start=(ct == 0), stop=(ct == CT - 1))
                    for tt in range(4):
                        nc.vector.tensor_tensor(
                            v_res[:, 4 * ic + tt, :], ps_v[tt][:], bvb_sb[:],
                            mybir.AluOpType.add)
                    return
                for tt in range(4):
                    ps = psA.tile([P, M], F32, tag="psA")
                    for ct in range(CT):
                        nc.tensor.matmul(
                            ps[:],
                            lhsT=x_cur[ct][:, tt * P:(tt + 1) * P],
                            rhs=wv_b[:, ct, :],
                            start=(ct == 0), stop=(ct == CT - 1))
                    nc.vector.tensor_tensor(
                        v_res[:, 4 * ic + tt, :], ps[:], bvb_sb[:],
                        mybir.AluOpType.add)

            if ic == 0:
                # wk lands last on the SP queue: fill the gap with proj v
                proj_v()
                proj_k()
            else:
                proj_k()
                proj_v()

            if ic + 1 < NT:
                x_next = load_x(ic + 1)

            # ---- attention for chunk ic (weaving ph3 of chunk ic-1) ----
            attn = atpool.tile([P, H_LOC, 512], F16, tag="attn")
            njt = 4 * ic + 4
            slots_left = H_LOC * njt
            for h in range(H_LOC):
                ps_av = psV.tile([P, 512], F32, tag="psV")
                exsum = espool.tile([P, 512], F16, tag="exsum")
                prev = None  # (ex tile, c0) awaiting its av matmul
                for jt in range(njt):
                    d = jt - 4 * ic
                    c0 = 128 * d if d > 0 else 0
                    ps_s = psS.tile([P, 512], F32, tag="psS")
                    nc.tensor.matmul(
                        ps_s[:, c0:],
                        lhsT=k_res[:, h, jt * P:(jt + 1) * P],
                        rhs=qcur[:, h, c0:],
                        start=True, stop=True)
                    ex = expool.tile([P, 512], F16, tag="ex")
                    nc.scalar.activation(
                        ex[:, c0:], ps_s[:, c0:],
                        mybir.ActivationFunctionType.Exp,
                        bias=eb_sb[:, 0:1], scale=SCALE)
                    if d >= 0:
                        # causal triangle: for every diagonal tile the global
                        # query base ts0+c0 equals the key base jt*P, so one
                        # [P,P] keep-where-col>=row mask serves them all
                        nc.vector.tensor_tensor(
                            ex[:, c0:c0 + P], ex[:, c0:c0 + P], tri_sb[:],
                            mybir.AluOpType.mult)
                    if pending is not None:
                        pending.step(1)
                    slots_left -= 1
                    if prev is not None:
                        pex, pc0, pjt = prev
                        nc.tensor.matmul(
                            ps_av[:, pc0:],
                            lhsT=v_res[:, pjt, h * HS:(h + 1) * HS],
                            rhs=pex[:, pc0:],
                            start=(pjt == 0), stop=False,
                            skip_group_check=True)
                    with nc.allow_low_precision(reason="fp16 softmax denom"):
                        if jt == 0:
                            if ic == 0:
                                nc.vector.tensor_copy(out=exsum[:], in_=ex[:])
                        elif jt == 1 and ic > 0:
                            nc.vector.tensor_tensor(
                                exsum[:], prev[0][:], ex[:],
                                mybir.AluOpType.add)
                        else:
                            nc.vector.tensor_tensor(
                                exsum[:, c0:], exsum[:, c0:], ex[:, c0:],
                                mybir.AluOpType.add)
                    prev = (ex, c0, jt)
                pex, pc0, pjt = prev
                nc.tensor.matmul(
                    ps_av[:, pc0:],
                    lhsT=v_res[:, pjt, h * HS:(h + 1) * HS],
                    rhs=pex[:, pc0:],
                    start=(pjt == 0), stop=True,
                    skip_group_check=True)
                rden = rdpool.tile([P, 512], F16, tag="rden")
                nc.gpsimd.partition_all_reduce(
                    rden[:], exsum[:], channels=P, reduce_op=bass_isa.ReduceOp.add)
                with nc.allow_low_precision(reason="softmax reciprocal"):
                    nc.vector.reciprocal(rden[:], rden[:])
                    nc.vector.tensor_tensor(
                        attn[:, h, :], ps_av[:], rden[:],
                        mybir.AluOpType.mult)

            if pending is not None:
                pending.finish()
            pending = Ph3(ic, attn,
                          pools=[psA, psS] if ic == NT - 1 else None)

        pending.finish()

    nc.finalize()
    return nc


def get_nc(phases=(1, 2, 3)):
    if phases not in _NC_CACHE:
        _NC_CACHE[phases] = _build(phases)
    return _NC_CACHE[phases]


def _rope_tables():
    inv_freq = 1.0 / (BASE ** (np.arange(0, ROT, 2, dtype=np.float64) / ROT))
    freqs = np.arange(T, dtype=np.float64)[:, None] * inv_freq[None, :]  # [T, 32]
    cos_h = np.cos(freqs).T.astype(np.float32)   # [32, T]
    sin_h = np.sin(freqs).T.astype(np.float32)
    cosT = np.concatenate([cos_h, cos_h], axis=0)          # [64, T]
    nsT = np.concatenate([-sin_h, sin_h], axis=0)          # [64, T] signed sin
    return (np.ascontiguousarray(cosT.astype(np.float16)),
            np.ascontiguousarray(nsT.astype(np.float16)))


def make_in_maps(x, Wq, bq, Wk, bk, Wv, bv, Wo, bo):
    cosT, nsT = _rope_tables()
    in_maps = []
    for c in range(N_CORES):
        b, g = divmod(c, TPG)
        ms = slice(g * M, (g + 1) * M)
        in_maps.append({
            "xh": np.ascontiguousarray(x[b].T.astype(np.float16)),
            "wqT": np.ascontiguousarray(Wq[ms].T.astype(np.float16)),
            "wkT": np.ascontiguousarray(Wk[ms].T.astype(np.float16)),
            "wvT": np.ascontiguousarray(Wv[ms].T.astype(np.float16)),
            "woT": np.ascontiguousarray(Wo[:, ms].T.astype(np.float16)),
            "bqc": np.ascontiguousarray(bq[ms].reshape(H_LOC, P).T),
            "bkc": np.ascontiguousarray(bk[ms].reshape(H_LOC, P).T),
            "bvr": np.ascontiguousarray(bv[ms].reshape(1, M)),
            "ebias": np.full((P, 1), EXP_BIAS, np.float32),
            "trimask": np.triu(np.ones((P, P), np.float16)),
            "cosT": cosT,
            "nsT": nsT,
        })
    return in_maps


def assemble(results, bo):
    out = np.empty((B, T, C), dtype=np.float32)
    for b in range(B):
        acc = results[b * TPG]["outT"].astype(np.float32)
        for g in range(1, TPG):
            acc = acc + results[b * TPG + g]["outT"].astype(np.float32)
        out[b] = acc.T + bo[None, :]
    return out


def kernel(x, Wq, bq, Wk, bk, Wv, bv, Wo, bo):
    nc = get_nc()
    in_maps = make_in_maps(np.asarray(x, np.float32),
                           np.asarray(Wq, np.float32), np.asarray(bq, np.float32),
                           np.asarray(Wk, np.float32), np.asarray(bk, np.float32),
                           np.asarray(Wv, np.float32), np.asarray(bv, np.float32),
                           np.asarray(Wo, np.float32), np.asarray(bo, np.float32))
    res = run_bass_kernel_spmd(nc, in_maps, list(range(N_CORES)))
    return assemble(res.results, np.asarray(bo, np.float32))


# revision 20
# speedup vs baseline: 1.1570x; 1.0072x over previous
"""Trainium2 Bass kernel for a full causal MHA layer (B=2, T=2048, C=2048, H=16,
partial RoPE on first 64 dims of each 128-dim head).

Sharding over 8 cores: core c handles batch b=c//4 and heads [4g, 4g+4), g=c%4
(tensor-parallel over heads x data-parallel over batch).

Fully fused single pass per core, fp16 data plane (fp32 PSUM accumulation):
  for each 512-token chunk ic:
    proj q/k (fp16 weights stationary, fp16 x moving), bias+partial-RoPE,
      q and k stay resident in SBUF (no DRAM spills)
    proj v -> v_res [key, jt, m] fp16 resident
    attention for chunk ic over heads h: per key-tile jt
      scoresT[k,q] (k_res stationary fp16, q moving fp16)
      -> exp(scale*s - 10*ln2) -> ex fp16 (Act), triangle mask on diagonal
         tiles only (DVE mult by a const [128,128] triu mask), exact causal
         col-trimming
      -> av accumulation outT[d,q] via PE; softmax denominator via DVE
         exsum adds + gpsimd partition_all_reduce (no PE ones-matmuls)
      output-projection matmuls of chunk ic-1 are woven between attention
      matmuls to keep PE busy during Act-latency windows
    phase3(ic): out partial outT[c,q] = sum_mt woT attn, DVE evict fp16, DMA
Host: slices inputs per core (fp16), sums the 4 TP partials per batch + bo.
"""

import math

import ml_dtypes
import numpy as np

NPF8 = ml_dtypes.float8_e4m3

import concourse.bass_isa as bass_isa
import concourse.mybir as mybir
import concourse.tile as tile
from concourse import bacc
from concourse.bass_utils import run_bass_kernel_spmd

F32 = mybir.dt.float32
F16 = mybir.dt.float16
F8 = mybir.dt.float8e4

B, T, C = 2, 2048, 2048
H = 16
HS = 128
ROT = 64
HALF = 32
BASE = 10000.0

N_CORES = 8
TPG = 4                # TP group size (heads split)
H_LOC = H // TPG       # 4 heads per core
M = H_LOC * HS         # 512 local head-dim columns
SCALE = 1.0 / math.sqrt(HS)
EXP_BIAS = -10.0 * math.log(2.0)   # exp(s*SCALE - 10ln2): keeps fp16 in range
SX = 16.0                 # fp8 quantization scale for x
SW = 1024.0               # fp8 quantization scale for Wq/Wk/Wv
INV_S = 1.0 / (SX * SW)   # folded into the projection evictions

P = 128
NT = T // 512          # 4 t-chunks of 512
CT = C // P            # 16 contraction tiles
JT = T // P            # 16 key tiles per head

_NC_CACHE = {}


def _build(phases=(1, 2, 3)):
    nc = bacc.Bacc(None, target_bir_lowering=False)

    xh8 = nc.declare_dram_parameter("xh8", [C, T], F8, isOutput=False)
    xh8r = nc.declare_dram_parameter("xh8r", [C, T], F8, isOutput=False)
    wq8 = nc.declare_dram_parameter("wq8", [C, M], F8, isOutput=False)
    wq8r = nc.declare_dram_parameter("wq8r", [C, M], F8, isOutput=False)
    wk8 = nc.declare_dram_parameter("wk8", [C, M], F8, isOutput=False)
    wk8r = nc.declare_dram_parameter("wk8r", [C, M], F8, isOutput=False)
    wv8 = nc.declare_dram_parameter("wv8", [C, M], F8, isOutput=False)
    wv8r = nc.declare_dram_parameter("wv8r", [C, M], F8, isOutput=False)
    woT = nc.declare_dram_parameter("woT", [M, C], F16, isOutput=False)
    bqc = nc.declare_dram_parameter("bqc", [P, H_LOC], F32, isOutput=False)
    bkc = nc.declare_dram_parameter("bkc", [P, H_LOC], F32, isOutput=False)
    bvr = nc.declare_dram_parameter("bvr", [1, M], F32, isOutput=False)
    ebias = nc.declare_dram_parameter("ebias", [P, 1], F32, isOutput=False)
    trimask = nc.declare_dram_parameter("trimask", [P, P], F16, isOutput=False)
    cosT = nc.declare_dram_parameter("cosT", [ROT, T], F16, isOutput=False)
    nsT = nc.declare_dram_parameter("nsT", [ROT, T], F16, isOutput=False)
    outT = nc.declare_dram_parameter("outT", [C, T], F16, isOutput=True)

    with tile.TileContext(nc) as tc, \
         tc.tile_pool(name="const", bufs=1) as const, \
         tc.tile_pool(name="xp", bufs=CT) as xpool, \
         tc.tile_pool(name="qc", bufs=2) as qpool, \
         tc.tile_pool(name="at", bufs=2) as atpool, \
         tc.tile_pool(name="rp", bufs=6) as rpool, \
         tc.tile_pool(name="exp", bufs=6) as expool, \
         tc.tile_pool(name="exs", bufs=2) as espool, \
         tc.tile_pool(name="rd", bufs=2) as rdpool, \
         tc.tile_pool(name="oe", bufs=3) as oepool, \
         tc.tile_pool(name="psA", bufs=3, space="PSUM") as psA, \
         tc.tile_pool(name="psS", bufs=3, space="PSUM") as psS, \
         tc.tile_pool(name="psV", bufs=2, space="PSUM") as psV:

        cos_sb = const.tile([ROT, T], F16, tag="cos")
        ns_sb = const.tile([ROT, T], F16, tag="ns")
        bq_sb = const.tile([P, H_LOC], F32, tag="bq")
        bk_sb = const.tile([P, H_LOC], F32, tag="bk")
        bv_sb = const.tile([1, M], F32, tag="bv")
        bvb_sb = const.tile([P, M], F32, tag="bvb")
        eb_sb = const.tile([P, 1], F32, tag="ebias")
        tri_sb = const.tile([P, P], F16, tag="trimask")
        k_res = const.tile([P, H_LOC, T], F16, tag="kres")
        v_res = const.tile([P, JT, M], F16, tag="vres")
        wq_t = [const.tile([P, CT, M], F8, tag=f"wq{i}", name=f"wq{i}")
                for i in range(2)]
        wk_t = [const.tile([P, CT, M], F8, tag=f"wk{i}", name=f"wk{i}")
                for i in range(2)]
        wv_t = [const.tile([P, CT, M], F8, tag=f"wv{i}", name=f"wv{i}")
                for i in range(2)]
        wo_b = const.tile([P, H_LOC, C], F16, tag="wob")
        wq_d = [wq8, wq8r]
        wk_d = [wk8, wk8r]
        wv_d = [wv8, wv8r]
        wre = [d[:].rearrange("(ct p) m -> p ct m", p=P)
               for d in (wq8, wq8r, wk8, wk8r, wv8, wv8r)]
        wqre, wqrre, wkre, wkrre, wvre, wvrre = wre
        wor = woT[:].rearrange("(mt p) c -> p mt c", p=P)

        xr = xh8[:].rearrange("(ct p) t -> p ct t", p=P)
        xrr = xh8r[:].rearrange("(ct p) t -> p ct t", p=P)
        otr = outT[:].rearrange("(co p) t -> p co t", p=P)

        def load_x(ic):
            ts0 = ic * 512
            xb = xpool.tile([P, CT, 512], F8, tag="xb8", name=f"xb{ic}", bufs=2)
            xbr = xpool.tile([P, CT, 512], F8, tag="xb8r", name=f"xbr{ic}",
                             bufs=2)
            for j in range(2):
                nc.sync.dma_start(out=xb[:, 8 * j:8 * j + 8, :],
                                  in_=xr[:, 8 * j:8 * j + 8, ts0:ts0 + 512])
                nc.sync.dma_start(out=xbr[:, 8 * j:8 * j + 8, :],
                                  in_=xrr[:, 8 * j:8 * j + 8, ts0:ts0 + 512])
            return xb, xbr

        # startup: x0 main+residual stream on the SP queue while wq pairs
        # stream on the Act queue; chunked so HWDGE overheads don't pace it.
        xb0 = xpool.tile([P, CT, 512], F8, tag="xb8", name="xb0", bufs=2)
        xb0r = xpool.tile([P, CT, 512], F8, tag="xb8r", name="xb0r", bufs=2)
        for j in range(4):
            a, b = 4 * j, 4 * j + 4
            nc.sync.dma_start(out=xb0[:, a:b, :], in_=xr[:, a:b, 0:512])
            nc.sync.dma_start(out=xb0r[:, a:b, :], in_=xrr[:, a:b, 0:512])
            nc.scalar.dma_start(out=wq_t[0][:, a:b, :], in_=wqre[:, a:b, :])
            nc.scalar.dma_start(out=wq_t[1][:, a:b, :], in_=wqrre[:, a:b, :])
        for j in range(2):
            a, b = 8 * j, 8 * j + 8
            nc.sync.dma_start(out=wv_t[0][:, a:b, :], in_=wvre[:, a:b, :])
            nc.sync.dma_start(out=wv_t[1][:, a:b, :], in_=wvrre[:, a:b, :])
        for j in range(2):
            a, b = 8 * j, 8 * j + 8
            nc.sync.dma_start(out=wk_t[0][:, a:b, :], in_=wkre[:, a:b, :])
            nc.sync.dma_start(out=wk_t[1][:, a:b, :], in_=wkrre[:, a:b, :])
        nc.gpsimd.dma_start(out=eb_sb[:], in_=ebias[:])
        nc.gpsimd.dma_start(out=tri_sb[:], in_=trimask[:])
        nc.gpsimd.dma_start(out=bq_sb[:], in_=bqc[:])
        nc.gpsimd.dma_start(out=bk_sb[:], in_=bkc[:])
        nc.gpsimd.dma_start(out=bv_sb[:], in_=bvr[:])
        nc.gpsimd.dma_start(out=cos_sb[:], in_=cosT[:])
        nc.gpsimd.dma_start(out=ns_sb[:], in_=nsT[:])
        nc.gpsimd.dma_start(out=bvb_sb[:], in_=bvr[0:1, :].to_broadcast([P, M]))

        NP = CT // 2   # 8 ct-pairs per contraction

        def fb_terms(w_t, xb, xbr):
            """(lhsT_tile, rhs_tile) per error-feedback term: main, w-res,
            x-res. All DoubleRow fp8 over ct-pairs."""
            return ((w_t[0], xb), (w_t[1], xb), (w_t[0], xbr))

        def rope_inplace(dst, tmp_src, ts0):
            """dst[0:ROT, 512] fp16 <- rope(tmp_src rows 0:ROT) in place.
            tmp_src rows are pre-rope biased values; dst may alias tmp_src."""
            sh = rpool.tile([ROT, 512], F16, tag="sh")
            nc.sync.dma_start(out=sh[0:HALF], in_=tmp_src[HALF:ROT])
            nc.sync.dma_start(out=sh[HALF:ROT], in_=tmp_src[0:HALF])
            rot = rpool.tile([ROT, 512], F16, tag="rot")
            nc.vector.tensor_tensor(rot[:], sh[:], ns_sb[:, ts0:ts0 + 512],
                                    mybir.AluOpType.mult)
            tcos = rpool.tile([ROT, 512], F16, tag="tcos")
            nc.vector.tensor_tensor(tcos[:], tmp_src[:ROT], cos_sb[:, ts0:ts0 + 512],
                                    mybir.AluOpType.mult)
            nc.vector.tensor_tensor(dst[0:ROT], tcos[:], rot[:],
                                    mybir.AluOpType.add)

        class Ph3:
            """Output projection for chunk ic; matmuls are dispensed one at a
            time (step) so they weave between attention matmuls."""

            def __init__(self, ic, attn, pools=None):
                self.ic = ic
                self.attn = attn
                self.items = [(co, mt) for co in range(CT) for mt in range(H_LOC)]
                self.pos = 0
                self.ps = None
                self.pools = pools or [psA]
                self.finishing = False

            def step(self, n=1):
                for _ in range(n):
                    if self.pos >= len(self.items):
                        return
                    co, mt = self.items[self.pos]
                    self.pos += 1
                    if mt == 0:
                        pool = self.pools[co % len(self.pools)]
                        self.ps = pool.tile([P, 512], F32,
                                            tag="psA" if pool is psA else "psS")
                    nc.tensor.matmul(
                        self.ps[:],
                        lhsT=wo_b[:, mt, co * P:(co + 1) * P],
                        rhs=self.attn[:, mt, :],
                        start=(mt == 0), stop=(mt == H_LOC - 1))
                    if mt == H_LOC - 1:
                        if co % 4 == 0:
                            self.ot = oepool.tile([P, 4, 512], F16, tag="ot")
                        if self.finishing and co % 2 == 1:
                            # post-attention block: DVE is draining attention
                            # tail work, so alternate evicts onto idle Act
                            nc.scalar.copy(self.ot[:, co % 4, :], self.ps[:])
                        else:
                            nc.vector.tensor_copy(out=self.ot[:, co % 4, :],
                                                  in_=self.ps[:])
                        last = self.ic == NT - 1
                        step = 2 if last else 4
                        if co % step == step - 1:
                            j0 = co % 4 - (step - 1)
                            nc.sync.dma_start(
                                out=otr[:, co - step + 1:co + 1,
                                        self.ic * 512:self.ic * 512 + 512],
                                in_=self.ot[:, j0:j0 + step, :])

            def finish(self):
                self.finishing = True
                self.step(len(self.items) - self.pos)

        pending = None

        for ic in range(NT):
            ts0 = ic * 512
            x_cur = (xb0, xb0r) if ic == 0 else x_next

            # ---- proj q ----
            qcur = qpool.tile([P, H_LOC, 512], F16, tag="qcur")
            xb8, xb8r = x_cur
            q_terms = fb_terms(wq_t, xb8, xb8r)
            if ic == 0:
                # pair-major with 4 concurrent PSUM groups: PE tracks the
                # x0/wq DMA chunk arrivals instead of stalling on one group
                ps_q = [psA.tile([P, 512], F32, tag="psA", name=f"psq{m}")
                        for m in range(3)]
                ps_q.append(psV.tile([P, 512], F32, tag="psV", name="psq3"))
                for a in range(NP):
                    for mt in range(H_LOC):
                        for ti, (wt, xt) in enumerate(q_terms):
                            nc.tensor.matmul(
                                ps_q[mt][:],
                                lhsT=wt[:, 2 * a:2 * a + 2,
                                        mt * P:(mt + 1) * P],
                                rhs=xt[:, 2 * a:2 * a + 2, :],
                                start=(a == 0 and ti == 0),
                                stop=(a == NP - 1 and ti == 2),
                                perf_mode=mybir.MatmulPerfMode.DoubleRow)
                for mt in range(H_LOC):
                    nc.scalar.activation(
                        qcur[:, mt, :], ps_q[mt][:],
                        mybir.ActivationFunctionType.Identity,
                        bias=bq_sb[:, mt:mt + 1], scale=INV_S)
                    rope_inplace(qcur[:, mt, :], qcur[:, mt, :], ts0)
            else:
                for mt in range(H_LOC):
                    ps = psA.tile([P, 512], F32, tag="psA")
                    for ti, (wt, xt) in enumerate(q_terms):
                        for a in range(NP):
                            nc.tensor.matmul(
                                ps[:],
                                lhsT=wt[:, 2 * a:2 * a + 2,
                                        mt * P:(mt + 1) * P],
                                rhs=xt[:, 2 * a:2 * a + 2, :],
                                start=(a == 0 and ti == 0),
                                stop=(a == NP - 1 and ti == 2),
                                perf_mode=mybir.MatmulPerfMode.DoubleRow)
                    nc.scalar.activation(
                        qcur[:, mt, :], ps[:],
                        mybir.ActivationFunctionType.Identity,
                        bias=bq_sb[:, mt:mt + 1], scale=INV_S)
                    rope_inplace(qcur[:, mt, :], qcur[:, mt, :], ts0)

            if ic == 0:
                for j in range(H_LOC):
                    nc.gpsimd.dma_start(out=wo_b[:, j:j + 1, :],
                                        in_=wor[:, j:j + 1, :])

            def proj_k():
                k_terms = fb_terms(wk_t, xb8, xb8r)
                for mt in range(H_LOC):
                    ps = psA.tile([P, 512], F32, tag="psA")
                    for ti, (wt, xt) in enumerate(k_terms):
                        for a in range(NP):
                            nc.tensor.matmul(
                                ps[:],
                                lhsT=wt[:, 2 * a:2 * a + 2,
                                        mt * P:(mt + 1) * P],
                                rhs=xt[:, 2 * a:2 * a + 2, :],
                                start=(a == 0 and ti == 0),
                                stop=(a == NP - 1 and ti == 2),
                                perf_mode=mybir.MatmulPerfMode.DoubleRow)
                    nc.scalar.activation(
                        k_res[ROT:P, mt, ts0:ts0 + 512], ps[ROT:P],
                        mybir.ActivationFunctionType.Identity,
                        bias=bk_sb[ROT:P, mt:mt + 1], scale=INV_S)
                    ktmp = rpool.tile([ROT, 512], F16, tag="ktmp")
                    nc.scalar.activation(
                        ktmp[:], ps[0:ROT],
                        mybir.ActivationFunctionType.Identity,
                        bias=bk_sb[0:ROT, mt:mt + 1], scale=INV_S)
                    rope_inplace(k_res[:, mt, ts0:ts0 + 512], ktmp[:], ts0)

            def proj_v():
                v_terms = ((xb8, wv_t[0]), (xb8, wv_t[1]), (xb8r, wv_t[0]))
                if ic == 0:
                    # pair-major: track the wv chunk arrivals during startup
                    ps_v = [psA.tile([P, M], F32, tag="psA", name=f"psv{t}")
                            for t in range(3)]
                    ps_v.append(psV.tile([P, M], F32, tag="psV", name="psv3"))
                    for a in range(NP):
                        for tt in range(4):
                            for ti, (xt, wt) in enumerate(v_terms):
                                nc.tensor.matmul(
                                    ps_v[tt][:],
                                    lhsT=xt[:, 2 * a:2 * a + 2,
                                            tt * P:(tt + 1) * P],
                                    rhs=wt[:, 2 * a:2 * a + 2, :],
                                    start=(a == 0 and ti == 0),
                                    stop=(a == NP - 1 and ti == 2),
                                    perf_mode=mybir.MatmulPerfMode.DoubleRow)
                    for tt in range(4):
                        nc.vector.scalar_tensor_tensor(
                            out=v_res[:, 4 * ic + tt, :], in0=ps_v[tt][:],
                            scalar=INV_S, in1=bvb_sb[:],
                            op0=mybir.AluOpType.mult, op1=mybir.AluOpType.add)
                    return
                for tt in range(4):
                    ps = psA.tile([P, M], F32, tag="psA")
                    for ti, (xt, wt) in enumerate(v_terms):
                        for a in range(NP):
                            nc.tensor.matmul(
                                ps[:],
                                lhsT=xt[:, 2 * a:2 * a + 2,
                                        tt * P:(tt + 1) * P],
                                rhs=wt[:, 2 * a:2 * a + 2, :],
                                start=(a == 0 and ti == 0),
                                stop=(a == NP - 1 and ti == 2),
                                perf_mode=mybir.MatmulPerfMode.DoubleRow)
                    nc.vector.scalar_tensor_tensor(
                        out=v_res[:, 4 * ic + tt, :], in0=ps[:],
                        scalar=INV_S, in1=bvb_sb[:],
                        op0=mybir.AluOpType.mult, op1=mybir.AluOpType.add)

            if ic == 0:
                # wk lands last on the SP queue: fill the gap with proj v
                proj_v()
                proj_k()
            else:
                proj_k()
                proj_v()

            if ic + 1 < NT:
                x_next = load_x(ic + 1)

            # ---- attention for chunk ic (weaving ph3 of chunk ic-1) ----
            attn = atpool.tile([P, H_LOC, 512], F16, tag="attn")
            njt = 4 * ic + 4
            slots_left = H_LOC * njt
            for h in range(H_LOC):
                ps_av = psV.tile([P, 512], F32, tag="psV")
                exsum = espool.tile([P, 512], F16, tag="exsum")
                prev = None  # (ex tile, c0) awaiting its av matmul
                for jt in range(njt):
                    d = jt - 4 * ic
                    c0 = 128 * d if d > 0 else 0
                    ps_s = psS.tile([P, 512], F32, tag="psS")
                    nc.tensor.matmul(
                        ps_s[:, c0:],
                        lhsT=k_res[:, h, jt * P:(jt + 1) * P],
                        rhs=qcur[:, h, c0:],
                        start=True, stop=True)
                    ex = expool.tile([P, 512], F16, tag="ex")
                    nc.scalar.activation(
                        ex[:, c0:], ps_s[:, c0:],
                        mybir.ActivationFunctionType.Exp,
                        bias=eb_sb[:, 0:1], scale=SCALE)
                    if d >= 0:
                        # causal triangle: for every diagonal tile the global
                        # query base ts0+c0 equals the key base jt*P, so one
                        # [P,P] keep-where-col>=row mask serves them all
                        nc.vector.tensor_tensor(
                            ex[:, c0:c0 + P], ex[:, c0:c0 + P], tri_sb[:],
                            mybir.AluOpType.mult)
                    if pending is not None:
                        pending.step(1)
                    slots_left -= 1
                    if prev is not None:
                        pex, pc0, pjt = prev
                        nc.tensor.matmul(
                            ps_av[:, pc0:],
                            lhsT=v_res[:, pjt, h * HS:(h + 1) * HS],
                            rhs=pex[:, pc0:],
                            start=(pjt == 0), stop=False,
                            skip_group_check=True)
                    with nc.allow_low_precision(reason="fp16 softmax denom"):
                        if jt == 0:
                            if ic == 0:
                                nc.vector.tensor_copy(out=exsum[:], in_=ex[:])
                        elif jt == 1 and ic > 0:
                            nc.vector.tensor_tensor(
                                exsum[:], prev[0][:], ex[:],
                                mybir.AluOpType.add)
                        else:
                            nc.vector.tensor_tensor(
                                exsum[:, c0:], exsum[:, c0:], ex[:, c0:],
                                mybir.AluOpType.add)
                    prev = (ex, c0, jt)
                pex, pc0, pjt = prev
                nc.tensor.matmul(
                    ps_av[:, pc0:],
                    lhsT=v_res[:, pjt, h * HS:(h + 1) * HS],
                    rhs=pex[:, pc0:],
                    start=(pjt == 0), stop=True,
                    skip_group_check=True)
                rden = rdpool.tile([P, 512], F16, tag="rden")
                nc.gpsimd.partition_all_reduce(
                    rden[:], exsum[:], channels=P, reduce_op=bass_isa.ReduceOp.add)
                with nc.allow_low_precision(reason="softmax reciprocal"):
                    nc.vector.reciprocal(rden[:], rden[:])
                    nc.vector.tensor_tensor(
                        attn[:, h, :], ps_av[:], rden[:],
                        mybir.AluOpType.mult)

            if pending is not None:
                pending.finish()
            pending = Ph3(ic, attn,
                          pools=[psA, psS] if ic == NT - 1 else None)

        pending.finish()

    nc.finalize()
    return nc


def get_nc(phases=(1, 2, 3)):
    if phases not in _NC_CACHE:
        _NC_CACHE[phases] = _build(phases)
    return _NC_CACHE[phases]


def _rope_tables():
    inv_freq = 1.0 / (BASE ** (np.arange(0, ROT, 2, dtype=np.float64) / ROT))
    freqs = np.arange(T, dtype=np.float64)[:, None] * inv_freq[None, :]  # [T, 32]
    cos_h = np.cos(freqs).T.astype(np.float32)   # [32, T]
    sin_h = np.sin(freqs).T.astype(np.float32)
    cosT = np.concatenate([cos_h, cos_h], axis=0)          # [64, T]
    nsT = np.concatenate([-sin_h, sin_h], axis=0)          # [64, T] signed sin
    return (np.ascontiguousarray(cosT.astype(np.float16)),
            np.ascontiguousarray(nsT.astype(np.float16)))


def _split8(a, s):
    """a*s = a8 + a8r (both fp8 e4m3) up to second-order quantization."""
    scaled = a * np.float32(s)
    a8 = scaled.astype(NPF8)
    a8r = (scaled - a8.astype(np.float32)).astype(NPF8)
    return np.ascontiguousarray(a8), np.ascontiguousarray(a8r)


def make_in_maps(x, Wq, bq, Wk, bk, Wv, bv, Wo, bo):
    cosT, nsT = _rope_tables()
    in_maps = []
    for c in range(N_CORES):
        b, g = divmod(c, TPG)
        ms = slice(g * M, (g + 1) * M)
        xh8, xh8r = _split8(x[b].T, SX)
        wq8, wq8r = _split8(Wq[ms].T, SW)
        wk8, wk8r = _split8(Wk[ms].T, SW)
        wv8, wv8r = _split8(Wv[ms].T, SW)
        in_maps.append({
            "xh8": xh8, "xh8r": xh8r,
            "wq8": wq8, "wq8r": wq8r,
            "wk8": wk8, "wk8r": wk8r,
            "wv8": wv8, "wv8r": wv8r,
            "woT": np.ascontiguousarray(Wo[:, ms].T.astype(np.float16)),
            "bqc": np.ascontiguousarray(bq[ms].reshape(H_LOC, P).T),
            "bkc": np.ascontiguousarray(bk[ms].reshape(H_LOC, P).T),
            "bvr": np.ascontiguousarray(bv[ms].reshape(1, M)),
            "ebias": np.full((P, 1), EXP_BIAS, np.float32),
            "trimask": np.triu(np.ones((P, P), np.float16)),
            "cosT": cosT,
            "nsT": nsT,
        })
    return in_maps


def assemble(results, bo):
    out = np.empty((B, T, C), dtype=np.float32)
    for b in range(B):
        acc = results[b * TPG]["outT"].astype(np.float32)
        for g in range(1, TPG):
            acc = acc + results[b * TPG + g]["outT"].astype(np.float32)
        out[b] = acc.T + bo[None, :]
    return out


def kernel(x, Wq, bq, Wk, bk, Wv, bv, Wo, bo):
    nc = get_nc()
    in_maps = make_in_maps(np.asarray(x, np.float32),
                           np.asarray(Wq, np.float32), np.asarray(bq, np.float32),
                           np.asarray(Wk, np.float32), np.asarray(bk, np.float32),
                           np.asarray(Wv, np.float32), np.asarray(bv, np.float32),
                           np.asarray(Wo, np.float32), np.asarray(bo, np.float32))
    res = run_bass_kernel_spmd(nc, in_maps, list(range(N_CORES)))
    return assemble(res.results, np.asarray(bo, np.float32))


# revision 21
# speedup vs baseline: 1.1641x; 1.0062x over previous
"""Trainium2 Bass kernel for a full causal MHA layer (B=2, T=2048, C=2048, H=16,
partial RoPE on first 64 dims of each 128-dim head).

Sharding over 8 cores: core c handles batch b=c//4 and heads [4g, 4g+4), g=c%4
(tensor-parallel over heads x data-parallel over batch).

Fully fused single pass per core, fp16 data plane (fp32 PSUM accumulation):
  for each 512-token chunk ic:
    proj q/k (fp16 weights stationary, fp16 x moving), bias+partial-RoPE,
      q and k stay resident in SBUF (no DRAM spills)
    proj v -> v_res [key, jt, m] fp16 resident
    attention for chunk ic over heads h: per key-tile jt
      scoresT[k,q] (k_res stationary fp16, q moving fp16)
      -> exp(scale*s - 10*ln2) -> ex fp16 (Act), triangle mask on diagonal
         tiles only (DVE mult by a const [128,128] triu mask), exact causal
         col-trimming
      -> av accumulation outT[d,q] via PE; softmax denominator via DVE
         exsum adds + gpsimd partition_all_reduce (no PE ones-matmuls)
      output-projection matmuls of chunk ic-1 are woven between attention
      matmuls to keep PE busy during Act-latency windows
    phase3(ic): out partial outT[c,q] = sum_mt woT attn, DVE evict fp16, DMA
Host: slices inputs per core (fp16), sums the 4 TP partials per batch + bo.
"""

import math

import ml_dtypes
import numpy as np

NPF8 = ml_dtypes.float8_e4m3

import concourse.bass_isa as bass_isa
import concourse.mybir as mybir
import concourse.tile as tile
from concourse import bacc
from concourse.bass_utils import run_bass_kernel_spmd

F32 = mybir.dt.float32
F16 = mybir.dt.float16
F8 = mybir.dt.float8e4

B, T, C = 2, 2048, 2048
H = 16
HS = 128
ROT = 64
HALF = 32
BASE = 10000.0

N_CORES = 8
TPG = 4                # TP group size (heads split)
H_LOC = H // TPG       # 4 heads per core
M = H_LOC * HS         # 512 local head-dim columns
SCALE = 1.0 / math.sqrt(HS)
EXP_BIAS = -10.0 * math.log(2.0)   # exp(s*SCALE - 10ln2): keeps fp16 in range
SX = 16.0                 # fp8 quantization scale for x
SW = 1024.0               # fp8 quantization scale for Wq/Wk/Wv
INV_S = 1.0 / (SX * SW)   # folded into the projection evictions

P = 128
NT = T // 512          # 4 t-chunks of 512
CT = C // P            # 16 contraction tiles
JT = T // P            # 16 key tiles per head

_NC_CACHE = {}


def _build(phases=(1, 2, 3)):
    nc = bacc.Bacc(None, target_bir_lowering=False)

    xh8 = nc.declare_dram_parameter("xh8", [C, T], F8, isOutput=False)
    xh8r = nc.declare_dram_parameter("xh8r", [C, T], F8, isOutput=False)
    wq8 = nc.declare_dram_parameter("wq8", [C, M], F8, isOutput=False)
    wq8r = nc.declare_dram_parameter("wq8r", [C, M], F8, isOutput=False)
    wk8 = nc.declare_dram_parameter("wk8", [C, M], F8, isOutput=False)
    wk8r = nc.declare_dram_parameter("wk8r", [C, M], F8, isOutput=False)
    wv8 = nc.declare_dram_parameter("wv8", [C, M], F8, isOutput=False)
    wv8r = nc.declare_dram_parameter("wv8r", [C, M], F8, isOutput=False)
    woT = nc.declare_dram_parameter("woT", [M, C], F16, isOutput=False)
    bqc = nc.declare_dram_parameter("bqc", [P, H_LOC], F32, isOutput=False)
    bkc = nc.declare_dram_parameter("bkc", [P, H_LOC], F32, isOutput=False)
    bvr = nc.declare_dram_parameter("bvr", [1, M], F32, isOutput=False)
    ebias = nc.declare_dram_parameter("ebias", [P, 1], F32, isOutput=False)
    trimask = nc.declare_dram_parameter("trimask", [P, P], F16, isOutput=False)
    cosT = nc.declare_dram_parameter("cosT", [ROT, T], F16, isOutput=False)
    nsT = nc.declare_dram_parameter("nsT", [ROT, T], F16, isOutput=False)
    outT = nc.declare_dram_parameter("outT", [C, T], F16, isOutput=True)

    with tile.TileContext(nc) as tc, \
         tc.tile_pool(name="const", bufs=1) as const, \
         tc.tile_pool(name="xp", bufs=CT) as xpool, \
         tc.tile_pool(name="qc", bufs=2) as qpool, \
         tc.tile_pool(name="at", bufs=2) as atpool, \
         tc.tile_pool(name="rp", bufs=6) as rpool, \
         tc.tile_pool(name="exp", bufs=6) as expool, \
         tc.tile_pool(name="exs", bufs=2) as espool, \
         tc.tile_pool(name="rd", bufs=2) as rdpool, \
         tc.tile_pool(name="oe", bufs=3) as oepool, \
         tc.tile_pool(name="psA", bufs=3, space="PSUM") as psA, \
         tc.tile_pool(name="psS", bufs=3, space="PSUM") as psS, \
         tc.tile_pool(name="psV", bufs=2, space="PSUM") as psV:

        cos_sb = const.tile([ROT, T], F16, tag="cos")
        ns_sb = const.tile([ROT, T], F16, tag="ns")
        bq_sb = const.tile([P, H_LOC], F32, tag="bq")
        bk_sb = const.tile([P, H_LOC], F32, tag="bk")
        bv_sb = const.tile([1, M], F32, tag="bv")
        bvb_sb = const.tile([P, M], F32, tag="bvb")
        eb_sb = const.tile([P, 1], F32, tag="ebias")
        tri_sb = const.tile([P, P], F16, tag="trimask")
        k_res = const.tile([P, H_LOC, T], F16, tag="kres")
        v_res = const.tile([P, JT, M], F16, tag="vres")
        wq_t = [const.tile([P, CT, M], F8, tag=f"wq{i}", name=f"wq{i}")
                for i in range(2)]
        wk_t = [const.tile([P, CT, M], F8, tag=f"wk{i}", name=f"wk{i}")
                for i in range(2)]
        wv_t = [const.tile([P, CT, M], F8, tag=f"wv{i}", name=f"wv{i}")
                for i in range(2)]
        wo_b = const.tile([P, H_LOC, C], F16, tag="wob")
        wq_d = [wq8, wq8r]
        wk_d = [wk8, wk8r]
        wv_d = [wv8, wv8r]
        wre = [d[:].rearrange("(ct p) m -> p ct m", p=P)
               for d in (wq8, wq8r, wk8, wk8r, wv8, wv8r)]
        wqre, wqrre, wkre, wkrre, wvre, wvrre = wre
        wor = woT[:].rearrange("(mt p) c -> p mt c", p=P)

        xr = xh8[:].rearrange("(ct p) t -> p ct t", p=P)
        xrr = xh8r[:].rearrange("(ct p) t -> p ct t", p=P)
        otr = outT[:].rearrange("(co p) t -> p co t", p=P)

        def load_x(ic):
            ts0 = ic * 512
            xb = xpool.tile([P, CT, 512], F8, tag="xb8", name=f"xb{ic}", bufs=2)
            xbr = xpool.tile([P, CT, 512], F8, tag="xb8r", name=f"xbr{ic}",
                             bufs=2)
            for j in range(2):
                nc.sync.dma_start(out=xb[:, 8 * j:8 * j + 8, :],
                                  in_=xr[:, 8 * j:8 * j + 8, ts0:ts0 + 512])
                nc.sync.dma_start(out=xbr[:, 8 * j:8 * j + 8, :],
                                  in_=xrr[:, 8 * j:8 * j + 8, ts0:ts0 + 512])
            return xb, xbr

        # startup: x0 main+residual stream on the SP queue while wq pairs
        # stream on the Act queue; chunked so HWDGE overheads don't pace it.
        xb0 = xpool.tile([P, CT, 512], F8, tag="xb8", name="xb0", bufs=2)
        xb0r = xpool.tile([P, CT, 512], F8, tag="xb8r", name="xb0r", bufs=2)
        for j in range(4):
            a, b = 4 * j, 4 * j + 4
            nc.sync.dma_start(out=xb0[:, a:b, :], in_=xr[:, a:b, 0:512])
            nc.sync.dma_start(out=xb0r[:, a:b, :], in_=xrr[:, a:b, 0:512])
            nc.scalar.dma_start(out=wq_t[0][:, a:b, :], in_=wqre[:, a:b, :])
            nc.scalar.dma_start(out=wq_t[1][:, a:b, :], in_=wqrre[:, a:b, :])
        for j in range(2):
            a, b = 8 * j, 8 * j + 8
            nc.sync.dma_start(out=wv_t[0][:, a:b, :], in_=wvre[:, a:b, :])
            nc.sync.dma_start(out=wv_t[1][:, a:b, :], in_=wvrre[:, a:b, :])
        for j in range(2):
            a, b = 8 * j, 8 * j + 8
            nc.sync.dma_start(out=wk_t[0][:, a:b, :], in_=wkre[:, a:b, :])
            nc.sync.dma_start(out=wk_t[1][:, a:b, :], in_=wkrre[:, a:b, :])
        nc.gpsimd.dma_start(out=eb_sb[:], in_=ebias[:])
        nc.gpsimd.dma_start(out=tri_sb[:], in_=trimask[:])
        nc.gpsimd.dma_start(out=bq_sb[:], in_=bqc[:])
        nc.gpsimd.dma_start(out=bk_sb[:], in_=bkc[:])
        nc.gpsimd.dma_start(out=bv_sb[:], in_=bvr[:])
        nc.gpsimd.dma_start(out=cos_sb[:], in_=cosT[:])
        nc.gpsimd.dma_start(out=ns_sb[:], in_=nsT[:])
        nc.gpsimd.dma_start(out=bvb_sb[:], in_=bvr[0:1, :].to_broadcast([P, M]))

        NP = CT // 2   # 8 ct-pairs per contraction

        def fb_terms(w_t, xb, xbr):
            """(lhsT_tile, rhs_tile) per error-feedback term: main, w-res,
            x-res. All DoubleRow fp8 over ct-pairs."""
            return ((w_t[0], xb), (w_t[1], xb), (w_t[0], xbr))

        def rope_inplace(dst, tmp_src, ts0):
            """dst[0:ROT, 512] fp16 <- rope(tmp_src rows 0:ROT) in place.
            tmp_src rows are pre-rope biased values; dst may alias tmp_src."""
            sh = rpool.tile([ROT, 512], F16, tag="sh")
            nc.sync.dma_start(out=sh[0:HALF], in_=tmp_src[HALF:ROT])
            nc.sync.dma_start(out=sh[HALF:ROT], in_=tmp_src[0:HALF])
            rot = rpool.tile([ROT, 512], F16, tag="rot")
            nc.vector.tensor_tensor(rot[:], sh[:], ns_sb[:, ts0:ts0 + 512],
                                    mybir.AluOpType.mult)
            tcos = rpool.tile([ROT, 512], F16, tag="tcos")
            nc.vector.tensor_tensor(tcos[:], tmp_src[:ROT], cos_sb[:, ts0:ts0 + 512],
                                    mybir.AluOpType.mult)
            nc.vector.tensor_tensor(dst[0:ROT], tcos[:], rot[:],
                                    mybir.AluOpType.add)

        class Ph3:
            """Output projection for chunk ic; matmuls are dispensed one at a
            time (step) so they weave between attention matmuls."""

            def __init__(self, ic, attn, pools=None):
                self.ic = ic
                self.attn = attn
                self.items = [(co, mt) for co in range(CT) for mt in range(H_LOC)]
                self.pos = 0
                self.ps = None
                self.pools = pools or [psA]
                self.finishing = False

            def step(self, n=1):
                for _ in range(n):
                    if self.pos >= len(self.items):
                        return
                    co, mt = self.items[self.pos]
                    self.pos += 1
                    if mt == 0:
                        pool = self.pools[co % len(self.pools)]
                        self.ps = pool.tile([P, 512], F32,
                                            tag="psA" if pool is psA else "psS")
                    nc.tensor.matmul(
                        self.ps[:],
                        lhsT=wo_b[:, mt, co * P:(co + 1) * P],
                        rhs=self.attn[:, mt, :],
                        start=(mt == 0), stop=(mt == H_LOC - 1))
                    if mt == H_LOC - 1:
                        if co % 4 == 0:
                            self.ot = oepool.tile([P, 4, 512], F16, tag="ot")
                        if self.finishing and co % 2 == 1:
                            # post-attention block: DVE is draining attention
                            # tail work, so alternate evicts onto idle Act
                            nc.scalar.copy(self.ot[:, co % 4, :], self.ps[:])
                        else:
                            nc.vector.tensor_copy(out=self.ot[:, co % 4, :],
                                                  in_=self.ps[:])
                        last = self.ic == NT - 1
                        step = 2 if last else 4
                        if co % step == step - 1:
                            j0 = co % 4 - (step - 1)
                            nc.sync.dma_start(
                                out=otr[:, co - step + 1:co + 1,
                                        self.ic * 512:self.ic * 512 + 512],
                                in_=self.ot[:, j0:j0 + step, :])

            def finish(self):
                self.finishing = True
                self.step(len(self.items) - self.pos)

        pending = None

        for ic in range(NT):
            ts0 = ic * 512
            x_cur = (xb0, xb0r) if ic == 0 else x_next

            # ---- proj q ----
            qcur = qpool.tile([P, H_LOC, 512], F16, tag="qcur")
            xb8, xb8r = x_cur
            q_terms = fb_terms(wq_t, xb8, xb8r)
            if ic == 0:
                # pair-major with 4 concurrent PSUM groups: PE tracks the
                # x0/wq DMA chunk arrivals instead of stalling on one group
                ps_q = [psA.tile([P, 512], F32, tag="psA", name=f"psq{m}")
                        for m in range(3)]
                ps_q.append(psV.tile([P, 512], F32, tag="psV", name="psq3"))
                for a in range(NP):
                    for mt in range(H_LOC):
                        for ti, (wt, xt) in enumerate(q_terms):
                            nc.tensor.matmul(
                                ps_q[mt][:],
                                lhsT=wt[:, 2 * a:2 * a + 2,
                                        mt * P:(mt + 1) * P],
                                rhs=xt[:, 2 * a:2 * a + 2, :],
                                start=(a == 0 and ti == 0),
                                stop=(a == NP - 1 and ti == 2),
                                perf_mode=mybir.MatmulPerfMode.DoubleRow)
                for mt in range(H_LOC):
                    nc.scalar.activation(
                        qcur[:, mt, :], ps_q[mt][:],
                        mybir.ActivationFunctionType.Identity,
                        bias=bq_sb[:, mt:mt + 1], scale=INV_S)
                    rope_inplace(qcur[:, mt, :], qcur[:, mt, :], ts0)
            else:
                for mt in range(H_LOC):
                    ps = psA.tile([P, 512], F32, tag="psA")
                    for ti, (wt, xt) in enumerate(q_terms):
                        for a in range(NP):
                            nc.tensor.matmul(
                                ps[:],
                                lhsT=wt[:, 2 * a:2 * a + 2,
                                        mt * P:(mt + 1) * P],
                                rhs=xt[:, 2 * a:2 * a + 2, :],
                                start=(a == 0 and ti == 0),
                                stop=(a == NP - 1 and ti == 2),
                                perf_mode=mybir.MatmulPerfMode.DoubleRow)
                    nc.scalar.activation(
                        qcur[:, mt, :], ps[:],
                        mybir.ActivationFunctionType.Identity,
                        bias=bq_sb[:, mt:mt + 1], scale=INV_S)
                    rope_inplace(qcur[:, mt, :], qcur[:, mt, :], ts0)

            if ic == 0:
                for j in range(H_LOC):
                    nc.gpsimd.dma_start(out=wo_b[:, j:j + 1, :],
                                        in_=wor[:, j:j + 1, :])

            def proj_k():
                k_terms = fb_terms(wk_t, xb8, xb8r)
                for mt in range(H_LOC):
                    ps = psA.tile([P, 512], F32, tag="psA")
                    for ti, (wt, xt) in enumerate(k_terms):
                        for a in range(NP):
                            nc.tensor.matmul(
                                ps[:],
                                lhsT=wt[:, 2 * a:2 * a + 2,
                                        mt * P:(mt + 1) * P],
                                rhs=xt[:, 2 * a:2 * a + 2, :],
                                start=(a == 0 and ti == 0),
                                stop=(a == NP - 1 and ti == 2),
                                perf_mode=mybir.MatmulPerfMode.DoubleRow)
                    nc.scalar.activation(
                        k_res[ROT:P, mt, ts0:ts0 + 512], ps[ROT:P],
                        mybir.ActivationFunctionType.Identity,
                        bias=bk_sb[ROT:P, mt:mt + 1], scale=INV_S)
                    ktmp = rpool.tile([ROT, 512], F16, tag="ktmp")
                    nc.scalar.activation(
                        ktmp[:], ps[0:ROT],
                        mybir.ActivationFunctionType.Identity,
                        bias=bk_sb[0:ROT, mt:mt + 1], scale=INV_S)
                    rope_inplace(k_res[:, mt, ts0:ts0 + 512], ktmp[:], ts0)

            def proj_v():
                v_terms = ((xb8, wv_t[0]), (xb8, wv_t[1]), (xb8r, wv_t[0]))
                if ic == 0:
                    # pair-major on psS (idle before attention): v does not
                    # wait for the q groups' psA banks to be evicted
                    ps_v = [psS.tile([P, M], F32, tag="psS", name=f"psv{t}")
                            for t in range(3)]
                    ps_v.append(psV.tile([P, M], F32, tag="psV", name="psv3"))
                    for a in range(NP):
                        for tt in range(4):
                            for ti, (xt, wt) in enumerate(v_terms):
                                nc.tensor.matmul(
                                    ps_v[tt][:],
                                    lhsT=xt[:, 2 * a:2 * a + 2,
                                            tt * P:(tt + 1) * P],
                                    rhs=wt[:, 2 * a:2 * a + 2, :],
                                    start=(a == 0 and ti == 0),
                                    stop=(a == NP - 1 and ti == 2),
                                    perf_mode=mybir.MatmulPerfMode.DoubleRow)
                    for tt in range(4):
                        nc.vector.scalar_tensor_tensor(
                            out=v_res[:, 4 * ic + tt, :], in0=ps_v[tt][:],
                            scalar=INV_S, in1=bvb_sb[:],
                            op0=mybir.AluOpType.mult, op1=mybir.AluOpType.add)
                    return
                for tt in range(4):
                    ps = psA.tile([P, M], F32, tag="psA")
                    for ti, (xt, wt) in enumerate(v_terms):
                        for a in range(NP):
                            nc.tensor.matmul(
                                ps[:],
                                lhsT=xt[:, 2 * a:2 * a + 2,
                                        tt * P:(tt + 1) * P],
                                rhs=wt[:, 2 * a:2 * a + 2, :],
                                start=(a == 0 and ti == 0),
                                stop=(a == NP - 1 and ti == 2),
                                perf_mode=mybir.MatmulPerfMode.DoubleRow)
                    nc.vector.scalar_tensor_tensor(
                        out=v_res[:, 4 * ic + tt, :], in0=ps[:],
                        scalar=INV_S, in1=bvb_sb[:],
                        op0=mybir.AluOpType.mult, op1=mybir.AluOpType.add)

            if ic == 0:
                # wk lands last on the SP queue: fill the gap with proj v
                proj_v()
                proj_k()
            else:
                proj_k()
                proj_v()

            if ic + 1 < NT:
                x_next = load_x(ic + 1)

            # ---- attention for chunk ic (weaving ph3 of chunk ic-1) ----
            attn = atpool.tile([P, H_LOC, 512], F16, tag="attn")
            njt = 4 * ic + 4
            slots_left = H_LOC * njt
            for h in range(H_LOC):
                ps_av = psV.tile([P, 512], F32, tag="psV")
                exsum = espool.tile([P, 512], F16, tag="exsum")
                prev = None  # (ex tile, c0) awaiting its av matmul
                for jt in range(njt):
                    d = jt - 4 * ic
                    c0 = 128 * d if d > 0 else 0
                    ps_s = psS.tile([P, 512], F32, tag="psS")
                    nc.tensor.matmul(
                        ps_s[:, c0:],
                        lhsT=k_res[:, h, jt * P:(jt + 1) * P],
                        rhs=qcur[:, h, c0:],
                        start=True, stop=True)
                    ex = expool.tile([P, 512], F16, tag="ex")
                    nc.scalar.activation(
                        ex[:, c0:], ps_s[:, c0:],
                        mybir.ActivationFunctionType.Exp,
                        bias=eb_sb[:, 0:1], scale=SCALE)
                    if d >= 0:
                        # causal triangle: for every diagonal tile the global
                        # query base ts0+c0 equals the key base jt*P, so one
                        # [P,P] keep-where-col>=row mask serves them all
                        nc.vector.tensor_tensor(
                            ex[:, c0:c0 + P], ex[:, c0:c0 + P], tri_sb[:],
                            mybir.AluOpType.mult)
                    if pending is not None:
                        pending.step(1)
                    slots_left -= 1
                    if prev is not None:
                        pex, pc0, pjt = prev
                        nc.tensor.matmul(
                            ps_av[:, pc0:],
                            lhsT=v_res[:, pjt, h * HS:(h + 1) * HS],
                            rhs=pex[:, pc0:],
                            start=(pjt == 0), stop=False,
                            skip_group_check=True)
                    with nc.allow_low_precision(reason="fp16 softmax denom"):
                        if jt == 0:
                            if ic == 0:
                                nc.vector.tensor_copy(out=exsum[:], in_=ex[:])
                        elif jt == 1 and ic > 0:
                            nc.vector.tensor_tensor(
                                exsum[:], prev[0][:], ex[:],
                                mybir.AluOpType.add)
                        else:
                            nc.vector.tensor_tensor(
                                exsum[:, c0:], exsum[:, c0:], ex[:, c0:],
                                mybir.AluOpType.add)
                    prev = (ex, c0, jt)
                pex, pc0, pjt = prev
                nc.tensor.matmul(
                    ps_av[:, pc0:],
                    lhsT=v_res[:, pjt, h * HS:(h + 1) * HS],
                    rhs=pex[:, pc0:],
                    start=(pjt == 0), stop=True,
                    skip_group_check=True)
                rden = rdpool.tile([P, 512], F16, tag="rden")
                nc.gpsimd.partition_all_reduce(
                    rden[:], exsum[:], channels=P, reduce_op=bass_isa.ReduceOp.add)
                with nc.allow_low_precision(reason="softmax reciprocal"):
                    nc.vector.reciprocal(rden[:], rden[:])
                    nc.vector.tensor_tensor(
                        attn[:, h, :], ps_av[:], rden[:],
                        mybir.AluOpType.mult)

            if pending is not None:
                pending.finish()
            pending = Ph3(ic, attn,
                          pools=[psA, psS] if ic == NT - 1 else None)

        pending.finish()

    nc.finalize()
    return nc


def get_nc(phases=(1, 2, 3)):
    if phases not in _NC_CACHE:
        _NC_CACHE[phases] = _build(phases)
    return _NC_CACHE[phases]


def _rope_tables():
    inv_freq = 1.0 / (BASE ** (np.arange(0, ROT, 2, dtype=np.float64) / ROT))
    freqs = np.arange(T, dtype=np.float64)[:, None] * inv_freq[None, :]  # [T, 32]
    cos_h = np.cos(freqs).T.astype(np.float32)   # [32, T]
    sin_h = np.sin(freqs).T.astype(np.float32)
    cosT = np.concatenate([cos_h, cos_h], axis=0)          # [64, T]
    nsT = np.concatenate([-sin_h, sin_h], axis=0)          # [64, T] signed sin
    return (np.ascontiguousarray(cosT.astype(np.float16)),
            np.ascontiguousarray(nsT.astype(np.float16)))


def _split8(a, s):
    """a*s = a8 + a8r (both fp8 e4m3) up to second-order quantization."""
    scaled = a * np.float32(s)
    a8 = scaled.astype(NPF8)
    a8r = (scaled - a8.astype(np.float32)).astype(NPF8)
    return np.ascontiguousarray(a8), np.ascontiguousarray(a8r)


def make_in_maps(x, Wq, bq, Wk, bk, Wv, bv, Wo, bo):
    cosT, nsT = _rope_tables()
    in_maps = []
    for c in range(N_CORES):
        b, g = divmod(c, TPG)
        ms = slice(g * M, (g + 1) * M)
        xh8, xh8r = _split8(x[b].T, SX)
        wq8, wq8r = _split8(Wq[ms].T, SW)
        wk8, wk8r = _split8(Wk[ms].T, SW)
        wv8, wv8r = _split8(Wv[ms].T, SW)
        in_maps.append({
            "xh8": xh8, "xh8r": xh8r,
            "wq8": wq8, "wq8r": wq8r,
            "wk8": wk8, "wk8r": wk8r,
            "wv8": wv8, "wv8r": wv8r,
            "woT": np.ascontiguousarray(Wo[:, ms].T.astype(np.float16)),
            "bqc": np.ascontiguousarray(bq[ms].reshape(H_LOC, P).T),
            "bkc": np.ascontiguousarray(bk[ms].reshape(H_LOC, P).T),
            "bvr": np.ascontiguousarray(bv[ms].reshape(1, M)),
            "ebias": np.full((P, 1), EXP_BIAS, np.float32),
            "trimask": np.triu(np.ones((P, P), np.float16)),
            "cosT": cosT,
            "nsT": nsT,
        })
    return in_maps


def assemble(results, bo):
    out = np.empty((B, T, C), dtype=np.float32)
    for b in range(B):
        acc = results[b * TPG]["outT"].astype(np.float32)
        for g in range(1, TPG):
            acc = acc + results[b * TPG + g]["outT"].astype(np.float32)
        out[b] = acc.T + bo[None, :]
    return out


def kernel(x, Wq, bq, Wk, bk, Wv, bv, Wo, bo):
    nc = get_nc()
    in_maps = make_in_maps(np.asarray(x, np.float32),
                           np.asarray(Wq, np.float32), np.asarray(bq, np.float32),
                           np.asarray(Wk, np.float32), np.asarray(bk, np.float32),
                           np.asarray(Wv, np.float32), np.asarray(bv, np.float32),
                           np.asarray(Wo, np.float32), np.asarray(bo, np.float32))
    res = run_bass_kernel_spmd(nc, in_maps, list(range(N_CORES)))
    return assemble(res.results, np.asarray(bo, np.float32))


# revision 22
# speedup vs baseline: 1.1642x; 1.0001x over previous
"""Trainium2 Bass kernel for a full causal MHA layer (B=2, T=2048, C=2048, H=16,
partial RoPE on first 64 dims of each 128-dim head).

Sharding over 8 cores: core c handles batch b=c//4 and heads [4g, 4g+4), g=c%4
(tensor-parallel over heads x data-parallel over batch).

Fully fused single pass per core, fp16 data plane (fp32 PSUM accumulation):
  for each 512-token chunk ic:
    proj q/k (fp16 weights stationary, fp16 x moving), bias+partial-RoPE,
      q and k stay resident in SBUF (no DRAM spills)
    proj v -> v_res [key, jt, m] fp16 resident
    attention for chunk ic over heads h: per key-tile jt
      scoresT[k,q] (k_res stationary fp16, q moving fp16)
      -> exp(scale*s - 10*ln2) -> ex fp16 (Act), triangle mask on diagonal
         tiles only (DVE mult by a const [128,128] triu mask), exact causal
         col-trimming
      -> av accumulation outT[d,q] via PE; softmax denominator via DVE
         exsum adds + gpsimd partition_all_reduce (no PE ones-matmuls)
      output-projection matmuls of chunk ic-1 are woven between attention
      matmuls to keep PE busy during Act-latency windows
    phase3(ic): out partial outT[c,q] = sum_mt woT attn, DVE evict fp16, DMA
Host: slices inputs per core (fp16), sums the 4 TP partials per batch + bo.
"""

import math

import ml_dtypes
import numpy as np

NPF8 = ml_dtypes.float8_e4m3

import concourse.bass_isa as bass_isa
import concourse.mybir as mybir
import concourse.tile as tile
from concourse import bacc
from concourse.bass_utils import run_bass_kernel_spmd

F32 = mybir.dt.float32
F16 = mybir.dt.float16
F8 = mybir.dt.float8e4

B, T, C = 2, 2048, 2048
H = 16
HS = 128
ROT = 64
HALF = 32
BASE = 10000.0

N_CORES = 8
TPG = 4                # TP group size (heads split)
H_LOC = H // TPG       # 4 heads per core
M = H_LOC * HS         # 512 local head-dim columns
SCALE = 1.0 / math.sqrt(HS)
EXP_BIAS = -10.0 * math.log(2.0)   # exp(s*SCALE - 10ln2): keeps fp16 in range
SX = 16.0                 # fp8 quantization scale for x
SW = 1024.0               # fp8 quantization scale for Wq/Wk/Wv
INV_S = 1.0 / (SX * SW)   # folded into the projection evictions

P = 128
NT = T // 512          # 4 t-chunks of 512
CT = C // P            # 16 contraction tiles
JT = T // P            # 16 key tiles per head

_NC_CACHE = {}


def _build(phases=(1, 2, 3)):
    nc = bacc.Bacc(None, target_bir_lowering=False)

    xh8 = nc.declare_dram_parameter("xh8", [C, T], F8, isOutput=False)
    xh8r = nc.declare_dram_parameter("xh8r", [C, T], F8, isOutput=False)
    wq8 = nc.declare_dram_parameter("wq8", [C, M], F8, isOutput=False)
    wq8r = nc.declare_dram_parameter("wq8r", [C, M], F8, isOutput=False)
    wk8 = nc.declare_dram_parameter("wk8", [C, M], F8, isOutput=False)
    wk8r = nc.declare_dram_parameter("wk8r", [C, M], F8, isOutput=False)
    wv8 = nc.declare_dram_parameter("wv8", [C, M], F8, isOutput=False)
    wv8r = nc.declare_dram_parameter("wv8r", [C, M], F8, isOutput=False)
    woT = nc.declare_dram_parameter("woT", [M, C], F16, isOutput=False)
    bqc = nc.declare_dram_parameter("bqc", [P, H_LOC], F32, isOutput=False)
    bkc = nc.declare_dram_parameter("bkc", [P, H_LOC], F32, isOutput=False)
    bvr = nc.declare_dram_parameter("bvr", [1, M], F32, isOutput=False)
    ebias = nc.declare_dram_parameter("ebias", [P, 1], F32, isOutput=False)
    trimask = nc.declare_dram_parameter("trimask", [P, P], F16, isOutput=False)
    cosT = nc.declare_dram_parameter("cosT", [ROT, T], F16, isOutput=False)
    nsT = nc.declare_dram_parameter("nsT", [ROT, T], F16, isOutput=False)
    outT = nc.declare_dram_parameter("outT", [C, T], F16, isOutput=True)

    with tile.TileContext(nc) as tc, \
         tc.tile_pool(name="const", bufs=1) as const, \
         tc.tile_pool(name="xp", bufs=CT) as xpool, \
         tc.tile_pool(name="qc", bufs=2) as qpool, \
         tc.tile_pool(name="at", bufs=2) as atpool, \
         tc.tile_pool(name="rp", bufs=6) as rpool, \
         tc.tile_pool(name="exp", bufs=6) as expool, \
         tc.tile_pool(name="exs", bufs=2) as espool, \
         tc.tile_pool(name="rd", bufs=2) as rdpool, \
         tc.tile_pool(name="oe", bufs=3) as oepool, \
         tc.tile_pool(name="psA", bufs=3, space="PSUM") as psA, \
         tc.tile_pool(name="psS", bufs=3, space="PSUM") as psS, \
         tc.tile_pool(name="psV", bufs=2, space="PSUM") as psV:

        cos_sb = const.tile([ROT, T], F16, tag="cos")
        ns_sb = const.tile([ROT, T], F16, tag="ns")
        bq_sb = const.tile([P, H_LOC], F32, tag="bq")
        bk_sb = const.tile([P, H_LOC], F32, tag="bk")
        bv_sb = const.tile([1, M], F32, tag="bv")
        bvb_sb = const.tile([P, M], F32, tag="bvb")
        eb_sb = const.tile([P, 1], F32, tag="ebias")
        tri_sb = const.tile([P, P], F16, tag="trimask")
        k_res = const.tile([P, H_LOC, T], F16, tag="kres")
        v_res = const.tile([P, JT, M], F16, tag="vres")
        wq_t = [const.tile([P, CT, M], F8, tag=f"wq{i}", name=f"wq{i}")
                for i in range(2)]
        wk_t = [const.tile([P, CT, M], F8, tag=f"wk{i}", name=f"wk{i}")
                for i in range(2)]
        wv_t = [const.tile([P, CT, M], F8, tag=f"wv{i}", name=f"wv{i}")
                for i in range(2)]
        wo_b = const.tile([P, H_LOC, C], F16, tag="wob")
        wq_d = [wq8, wq8r]
        wk_d = [wk8, wk8r]
        wv_d = [wv8, wv8r]
        wre = [d[:].rearrange("(ct p) m -> p ct m", p=P)
               for d in (wq8, wq8r, wk8, wk8r, wv8, wv8r)]
        wqre, wqrre, wkre, wkrre, wvre, wvrre = wre
        wor = woT[:].rearrange("(mt p) c -> p mt c", p=P)

        xr = xh8[:].rearrange("(ct p) t -> p ct t", p=P)
        xrr = xh8r[:].rearrange("(ct p) t -> p ct t", p=P)
        otr = outT[:].rearrange("(co p) t -> p co t", p=P)

        def load_x(ic):
            ts0 = ic * 512
            xb = xpool.tile([P, CT, 512], F8, tag="xb8", name=f"xb{ic}", bufs=2)
            xbr = xpool.tile([P, CT, 512], F8, tag="xb8r", name=f"xbr{ic}",
                             bufs=2)
            for j in range(2):
                nc.sync.dma_start(out=xb[:, 8 * j:8 * j + 8, :],
                                  in_=xr[:, 8 * j:8 * j + 8, ts0:ts0 + 512])
                nc.sync.dma_start(out=xbr[:, 8 * j:8 * j + 8, :],
                                  in_=xrr[:, 8 * j:8 * j + 8, ts0:ts0 + 512])
            return xb, xbr

        # startup: x0 main+residual stream on the SP queue while wq pairs
        # stream on the Act queue; chunked so HWDGE overheads don't pace it.
        xb0 = xpool.tile([P, CT, 512], F8, tag="xb8", name="xb0", bufs=2)
        xb0r = xpool.tile([P, CT, 512], F8, tag="xb8r", name="xb0r", bufs=2)
        for j in range(4):
            a, b = 4 * j, 4 * j + 4
            nc.sync.dma_start(out=xb0[:, a:b, :], in_=xr[:, a:b, 0:512])
            nc.sync.dma_start(out=xb0r[:, a:b, :], in_=xrr[:, a:b, 0:512])
            nc.scalar.dma_start(out=wq_t[0][:, a:b, :], in_=wqre[:, a:b, :])
            nc.scalar.dma_start(out=wq_t[1][:, a:b, :], in_=wqrre[:, a:b, :])
        for j in range(2):
            a, b = 8 * j, 8 * j + 8
            nc.sync.dma_start(out=wv_t[0][:, a:b, :], in_=wvre[:, a:b, :])
        for j in range(2):
            a, b = 8 * j, 8 * j + 8
            nc.sync.dma_start(out=wv_t[1][:, a:b, :], in_=wvrre[:, a:b, :])
        for j in range(2):
            a, b = 8 * j, 8 * j + 8
            nc.sync.dma_start(out=wk_t[0][:, a:b, :], in_=wkre[:, a:b, :])
        for j in range(2):
            a, b = 8 * j, 8 * j + 8
            nc.sync.dma_start(out=wk_t[1][:, a:b, :], in_=wkrre[:, a:b, :])
        nc.gpsimd.dma_start(out=eb_sb[:], in_=ebias[:])
        nc.gpsimd.dma_start(out=tri_sb[:], in_=trimask[:])
        nc.gpsimd.dma_start(out=bq_sb[:], in_=bqc[:])
        nc.gpsimd.dma_start(out=bk_sb[:], in_=bkc[:])
        nc.gpsimd.dma_start(out=bv_sb[:], in_=bvr[:])
        nc.gpsimd.dma_start(out=cos_sb[:], in_=cosT[:])
        nc.gpsimd.dma_start(out=ns_sb[:], in_=nsT[:])
        nc.gpsimd.dma_start(out=bvb_sb[:], in_=bvr[0:1, :].to_broadcast([P, M]))

        NP = CT // 2   # 8 ct-pairs per contraction

        def fb_terms(w_t, xb, xbr):
            """(lhsT_tile, rhs_tile) per error-feedback term: main, w-res,
            x-res. All DoubleRow fp8 over ct-pairs."""
            return ((w_t[0], xb), (w_t[1], xb), (w_t[0], xbr))

        def rope_inplace(dst, tmp_src, ts0):
            """dst[0:ROT, 512] fp16 <- rope(tmp_src rows 0:ROT) in place.
            tmp_src rows are pre-rope biased values; dst may alias tmp_src."""
            sh = rpool.tile([ROT, 512], F16, tag="sh")
            nc.sync.dma_start(out=sh[0:HALF], in_=tmp_src[HALF:ROT])
            nc.sync.dma_start(out=sh[HALF:ROT], in_=tmp_src[0:HALF])
            rot = rpool.tile([ROT, 512], F16, tag="rot")
            nc.vector.tensor_tensor(rot[:], sh[:], ns_sb[:, ts0:ts0 + 512],
                                    mybir.AluOpType.mult)
            tcos = rpool.tile([ROT, 512], F16, tag="tcos")
            nc.vector.tensor_tensor(tcos[:], tmp_src[:ROT], cos_sb[:, ts0:ts0 + 512],
                                    mybir.AluOpType.mult)
            nc.vector.tensor_tensor(dst[0:ROT], tcos[:], rot[:],
                                    mybir.AluOpType.add)

        class Ph3:
            """Output projection for chunk ic; matmuls are dispensed one at a
            time (step) so they weave between attention matmuls."""

            def __init__(self, ic, attn, pools=None):
                self.ic = ic
                self.attn = attn
                self.items = [(co, mt) for co in range(CT) for mt in range(H_LOC)]
                self.pos = 0
                self.ps = None
                self.pools = pools or [psA]
                self.finishing = False

            def step(self, n=1):
                for _ in range(n):
                    if self.pos >= len(self.items):
                        return
                    co, mt = self.items[self.pos]
                    self.pos += 1
                    if mt == 0:
                        pool = self.pools[co % len(self.pools)]
                        self.ps = pool.tile([P, 512], F32,
                                            tag="psA" if pool is psA else "psS")
                    nc.tensor.matmul(
                        self.ps[:],
                        lhsT=wo_b[:, mt, co * P:(co + 1) * P],
                        rhs=self.attn[:, mt, :],
                        start=(mt == 0), stop=(mt == H_LOC - 1))
                    if mt == H_LOC - 1:
                        if co % 4 == 0:
                            self.ot = oepool.tile([P, 4, 512], F16, tag="ot")
                        if self.finishing and co % 2 == 1:
                            # post-attention block: DVE is draining attention
                            # tail work, so alternate evicts onto idle Act
                            nc.scalar.copy(self.ot[:, co % 4, :], self.ps[:])
                        else:
                            nc.vector.tensor_copy(out=self.ot[:, co % 4, :],
                                                  in_=self.ps[:])
                        last = self.ic == NT - 1
                        step = 2 if last else 4
                        if co % step == step - 1:
                            j0 = co % 4 - (step - 1)
                            nc.sync.dma_start(
                                out=otr[:, co - step + 1:co + 1,
                                        self.ic * 512:self.ic * 512 + 512],
                                in_=self.ot[:, j0:j0 + step, :])

            def finish(self):
                self.finishing = True
                self.step(len(self.items) - self.pos)

        pending = None

        for ic in range(NT):
            ts0 = ic * 512
            x_cur = (xb0, xb0r) if ic == 0 else x_next

            # ---- proj q ----
            qcur = qpool.tile([P, H_LOC, 512], F16, tag="qcur")
            xb8, xb8r = x_cur
            q_terms = fb_terms(wq_t, xb8, xb8r)
            if ic == 0:
                # pair-major with 4 concurrent PSUM groups: PE tracks the
                # x0/wq DMA chunk arrivals instead of stalling on one group
                ps_q = [psA.tile([P, 512], F32, tag="psA", name=f"psq{m}")
                        for m in range(3)]
                ps_q.append(psV.tile([P, 512], F32, tag="psV", name="psq3"))
                for a in range(NP):
                    for mt in range(H_LOC):
                        for ti, (wt, xt) in enumerate(q_terms):
                            nc.tensor.matmul(
                                ps_q[mt][:],
                                lhsT=wt[:, 2 * a:2 * a + 2,
                                        mt * P:(mt + 1) * P],
                                rhs=xt[:, 2 * a:2 * a + 2, :],
                                start=(a == 0 and ti == 0),
                                stop=(a == NP - 1 and ti == 2),
                                perf_mode=mybir.MatmulPerfMode.DoubleRow)
                for mt in range(H_LOC):
                    nc.scalar.activation(
                        qcur[:, mt, :], ps_q[mt][:],
                        mybir.ActivationFunctionType.Identity,
                        bias=bq_sb[:, mt:mt + 1], scale=INV_S)
                    rope_inplace(qcur[:, mt, :], qcur[:, mt, :], ts0)
            else:
                for mt in range(H_LOC):
                    ps = psA.tile([P, 512], F32, tag="psA")
                    for ti, (wt, xt) in enumerate(q_terms):
                        for a in range(NP):
                            nc.tensor.matmul(
                                ps[:],
                                lhsT=wt[:, 2 * a:2 * a + 2,
                                        mt * P:(mt + 1) * P],
                                rhs=xt[:, 2 * a:2 * a + 2, :],
                                start=(a == 0 and ti == 0),
                                stop=(a == NP - 1 and ti == 2),
                                perf_mode=mybir.MatmulPerfMode.DoubleRow)
                    nc.scalar.activation(
                        qcur[:, mt, :], ps[:],
                        mybir.ActivationFunctionType.Identity,
                        bias=bq_sb[:, mt:mt + 1], scale=INV_S)
                    rope_inplace(qcur[:, mt, :], qcur[:, mt, :], ts0)

            if ic == 0:
                for j in range(H_LOC):
                    nc.gpsimd.dma_start(out=wo_b[:, j:j + 1, :],
                                        in_=wor[:, j:j + 1, :])

            def proj_k():
                if ic == 0:
                    # w-residual term last: wk8r chunks are the last arrivals
                    k_terms = ((wk_t[0], xb8), (wk_t[0], xb8r), (wk_t[1], xb8))
                else:
                    k_terms = fb_terms(wk_t, xb8, xb8r)
                for mt in range(H_LOC):
                    ps = psA.tile([P, 512], F32, tag="psA")
                    for ti, (wt, xt) in enumerate(k_terms):
                        for a in range(NP):
                            nc.tensor.matmul(
                                ps[:],
                                lhsT=wt[:, 2 * a:2 * a + 2,
                                        mt * P:(mt + 1) * P],
                                rhs=xt[:, 2 * a:2 * a + 2, :],
                                start=(a == 0 and ti == 0),
                                stop=(a == NP - 1 and ti == 2),
                                perf_mode=mybir.MatmulPerfMode.DoubleRow)
                    nc.scalar.activation(
                        k_res[ROT:P, mt, ts0:ts0 + 512], ps[ROT:P],
                        mybir.ActivationFunctionType.Identity,
                        bias=bk_sb[ROT:P, mt:mt + 1], scale=INV_S)
                    ktmp = rpool.tile([ROT, 512], F16, tag="ktmp")
                    nc.scalar.activation(
                        ktmp[:], ps[0:ROT],
                        mybir.ActivationFunctionType.Identity,
                        bias=bk_sb[0:ROT, mt:mt + 1], scale=INV_S)
                    rope_inplace(k_res[:, mt, ts0:ts0 + 512], ktmp[:], ts0)

            def proj_v():
                v_terms = ((xb8, wv_t[0]), (xb8, wv_t[1]), (xb8r, wv_t[0]))
                if ic == 0:
                    # pair-major on psS (idle before attention); term-major
                    # with the w-residual term last, since the residual
                    # weight chunks are the last DMAs to land
                    ps_v = [psS.tile([P, M], F32, tag="psS", name=f"psv{t}")
                            for t in range(3)]
                    ps_v.append(psV.tile([P, M], F32, tag="psV", name="psv3"))
                    vt0 = ((xb8, wv_t[0]), (xb8r, wv_t[0]), (xb8, wv_t[1]))
                    for ti, (xt, wt) in enumerate(vt0):
                        for a in range(NP):
                            for tt in range(4):
                                nc.tensor.matmul(
                                    ps_v[tt][:],
                                    lhsT=xt[:, 2 * a:2 * a + 2,
                                            tt * P:(tt + 1) * P],
                                    rhs=wt[:, 2 * a:2 * a + 2, :],
                                    start=(a == 0 and ti == 0),
                                    stop=(a == NP - 1 and ti == 2),
                                    perf_mode=mybir.MatmulPerfMode.DoubleRow)
                    for tt in range(4):
                        nc.vector.scalar_tensor_tensor(
                            out=v_res[:, 4 * ic + tt, :], in0=ps_v[tt][:],
                            scalar=INV_S, in1=bvb_sb[:],
                            op0=mybir.AluOpType.mult, op1=mybir.AluOpType.add)
                    return
                for tt in range(4):
                    ps = psA.tile([P, M], F32, tag="psA")
                    for ti, (xt, wt) in enumerate(v_terms):
                        for a in range(NP):
                            nc.tensor.matmul(
                                ps[:],
                                lhsT=xt[:, 2 * a:2 * a + 2,
                                        tt * P:(tt + 1) * P],
                                rhs=wt[:, 2 * a:2 * a + 2, :],
                                start=(a == 0 and ti == 0),
                                stop=(a == NP - 1 and ti == 2),
                                perf_mode=mybir.MatmulPerfMode.DoubleRow)
                    nc.vector.scalar_tensor_tensor(
                        out=v_res[:, 4 * ic + tt, :], in0=ps[:],
                        scalar=INV_S, in1=bvb_sb[:],
                        op0=mybir.AluOpType.mult, op1=mybir.AluOpType.add)

            if ic == 0:
                # wk lands last on the SP queue: fill the gap with proj v
                proj_v()
                proj_k()
            else:
                proj_k()
                proj_v()

            if ic + 1 < NT:
                x_next = load_x(ic + 1)

            # ---- attention for chunk ic (weaving ph3 of chunk ic-1) ----
            attn = atpool.tile([P, H_LOC, 512], F16, tag="attn")
            njt = 4 * ic + 4
            slots_left = H_LOC * njt
            for h in range(H_LOC):
                ps_av = psV.tile([P, 512], F32, tag="psV")
                exsum = espool.tile([P, 512], F16, tag="exsum")
                prev = None  # (ex tile, c0) awaiting its av matmul
                for jt in range(njt):
                    d = jt - 4 * ic
                    c0 = 128 * d if d > 0 else 0
                    ps_s = psS.tile([P, 512], F32, tag="psS")
                    nc.tensor.matmul(
                        ps_s[:, c0:],
                        lhsT=k_res[:, h, jt * P:(jt + 1) * P],
                        rhs=qcur[:, h, c0:],
                        start=True, stop=True)
                    ex = expool.tile([P, 512], F16, tag="ex")
                    nc.scalar.activation(
                        ex[:, c0:], ps_s[:, c0:],
                        mybir.ActivationFunctionType.Exp,
                        bias=eb_sb[:, 0:1], scale=SCALE)
                    if d >= 0:
                        # causal triangle: for every diagonal tile the global
                        # query base ts0+c0 equals the key base jt*P, so one
                        # [P,P] keep-where-col>=row mask serves them all
                        nc.vector.tensor_tensor(
                            ex[:, c0:c0 + P], ex[:, c0:c0 + P], tri_sb[:],
                            mybir.AluOpType.mult)
                    if pending is not None:
                        pending.step(1)
                    slots_left -= 1
                    if prev is not None:
                        pex, pc0, pjt = prev
                        nc.tensor.matmul(
                            ps_av[:, pc0:],
                            lhsT=v_res[:, pjt, h * HS:(h + 1) * HS],
                            rhs=pex[:, pc0:],
                            start=(pjt == 0), stop=False,
                            skip_group_check=True)
                    with nc.allow_low_precision(reason="fp16 softmax denom"):
                        if jt == 0:
                            if ic == 0:
                                nc.vector.tensor_copy(out=exsum[:], in_=ex[:])
                        elif jt == 1 and ic > 0:
                            nc.vector.tensor_tensor(
                                exsum[:], prev[0][:], ex[:],
                                mybir.AluOpType.add)
                        else:
                            nc.vector.tensor_tensor(
                                exsum[:, c0:], exsum[:, c0:], ex[:, c0:],
                                mybir.AluOpType.add)
                    prev = (ex, c0, jt)
                pex, pc0, pjt = prev
                nc.tensor.matmul(
                    ps_av[:, pc0:],
                    lhsT=v_res[:, pjt, h * HS:(h + 1) * HS],
                    rhs=pex[:, pc0:],
                    start=(pjt == 0), stop=True,
                    skip_group_check=True)
                rden = rdpool.tile([P, 512], F16, tag="rden")
                nc.gpsimd.partition_all_reduce(
                    rden[:], exsum[:], channels=P, reduce_op=bass_isa.ReduceOp.add)
                with nc.allow_low_precision(reason="softmax reciprocal"):
                    nc.vector.reciprocal(rden[:], rden[:])
                    nc.vector.tensor_tensor(
                        attn[:, h, :], ps_av[:], rden[:],
                        mybir.AluOpType.mult)

            if pending is not None:
                pending.finish()
            pending = Ph3(ic, attn,
                          pools=[psA, psS] if ic == NT - 1 else None)

        pending.finish()

    nc.finalize()
    return nc


def get_nc(phases=(1, 2, 3)):
    if phases not in _NC_CACHE:
        _NC_CACHE[phases] = _build(phases)
    return _NC_CACHE[phases]


def _rope_tables():
    inv_freq = 1.0 / (BASE ** (np.arange(0, ROT, 2, dtype=np.float64) / ROT))
    freqs = np.arange(T, dtype=np.float64)[:, None] * inv_freq[None, :]  # [T, 32]
    cos_h = np.cos(freqs).T.astype(np.float32)   # [32, T]
    sin_h = np.sin(freqs).T.astype(np.float32)
    cosT = np.concatenate([cos_h, cos_h], axis=0)          # [64, T]
    nsT = np.concatenate([-sin_h, sin_h], axis=0)          # [64, T] signed sin
    return (np.ascontiguousarray(cosT.astype(np.float16)),
            np.ascontiguousarray(nsT.astype(np.float16)))


def _split8(a, s):
    """a*s = a8 + a8r (both fp8 e4m3) up to second-order quantization."""
    scaled = a * np.float32(s)
    a8 = scaled.astype(NPF8)
    a8r = (scaled - a8.astype(np.float32)).astype(NPF8)
    return np.ascontiguousarray(a8), np.ascontiguousarray(a8r)


def make_in_maps(x, Wq, bq, Wk, bk, Wv, bv, Wo, bo):
    cosT, nsT = _rope_tables()
    in_maps = []
    for c in range(N_CORES):
        b, g = divmod(c, TPG)
        ms = slice(g * M, (g + 1) * M)
        xh8, xh8r = _split8(x[b].T, SX)
        wq8, wq8r = _split8(Wq[ms].T, SW)
        wk8, wk8r = _split8(Wk[ms].T, SW)
        wv8, wv8r = _split8(Wv[ms].T, SW)
        in_maps.append({
            "xh8": xh8, "xh8r": xh8r,
            "wq8": wq8, "wq8r": wq8r,
            "wk8": wk8, "wk8r": wk8r,
            "wv8": wv8, "wv8r": wv8r,
            "woT": np.ascontiguousarray(Wo[:, ms].T.astype(np.float16)),
            "bqc": np.ascontiguousarray(bq[ms].reshape(H_LOC, P).T),
            "bkc": np.ascontiguousarray(bk[ms].reshape(H_LOC, P).T),
            "bvr": np.ascontiguousarray(bv[ms].reshape(1, M)),
            "ebias": np.full((P, 1), EXP_BIAS, np.float32),
            "trimask": np.triu(np.ones((P, P), np.float16)),
            "cosT": cosT,
            "nsT": nsT,
        })
    return in_maps


def assemble(results, bo):
    out = np.empty((B, T, C), dtype=np.float32)
    for b in range(B):
        acc = results[b * TPG]["outT"].astype(np.float32)
        for g in range(1, TPG):
            acc = acc + results[b * TPG + g]["outT"].astype(np.float32)
        out[b] = acc.T + bo[None, :]
    return out


def kernel(x, Wq, bq, Wk, bk, Wv, bv, Wo, bo):
    nc = get_nc()
    in_maps = make_in_maps(np.asarray(x, np.float32),
                           np.asarray(Wq, np.float32), np.asarray(bq, np.float32),
                           np.asarray(Wk, np.float32), np.asarray(bk, np.float32),
                           np.asarray(Wv, np.float32), np.asarray(bv, np.float32),
                           np.asarray(Wo, np.float32), np.asarray(bo, np.float32))
    res = run_bass_kernel_spmd(nc, in_maps, list(range(N_CORES)))
    return assemble(res.results, np.asarray(bo, np.float32))


# revision 23
# speedup vs baseline: 1.1643x; 1.0001x over previous
"""Trainium2 Bass kernel for a full causal MHA layer (B=2, T=2048, C=2048, H=16,
partial RoPE on first 64 dims of each 128-dim head).

Sharding over 8 cores: core c handles batch b=c//4 and heads [4g, 4g+4), g=c%4
(tensor-parallel over heads x data-parallel over batch).

Fully fused single pass per core, fp16 data plane (fp32 PSUM accumulation):
  for each 512-token chunk ic:
    proj q/k (fp16 weights stationary, fp16 x moving), bias+partial-RoPE,
      q and k stay resident in SBUF (no DRAM spills)
    proj v -> v_res [key, jt, m] fp16 resident
    attention for chunk ic over heads h: per key-tile jt
      scoresT[k,q] (k_res stationary fp16, q moving fp16)
      -> exp(scale*s - 10*ln2) -> ex fp16 (Act), triangle mask on diagonal
         tiles only (DVE mult by a const [128,128] triu mask), exact causal
         col-trimming
      -> av accumulation outT[d,q] via PE; softmax denominator via DVE
         exsum adds + gpsimd partition_all_reduce (no PE ones-matmuls)
      output-projection matmuls of chunk ic-1 are woven between attention
      matmuls to keep PE busy during Act-latency windows
    phase3(ic): out partial outT[c,q] = sum_mt woT attn, DVE evict fp16, DMA
Host: slices inputs per core (fp16), sums the 4 TP partials per batch + bo.
"""

import math

import ml_dtypes
import numpy as np

NPF8 = ml_dtypes.float8_e4m3

import concourse.bass_isa as bass_isa
import concourse.mybir as mybir
import concourse.tile as tile
from concourse import bacc
from concourse.bass_utils import run_bass_kernel_spmd

F32 = mybir.dt.float32
F16 = mybir.dt.float16
F8 = mybir.dt.float8e4

B, T, C = 2, 2048, 2048
H = 16
HS = 128
ROT = 64
HALF = 32
BASE = 10000.0

N_CORES = 8
TPG = 4                # TP group size (heads split)
H_LOC = H // TPG       # 4 heads per core
M = H_LOC * HS         # 512 local head-dim columns
SCALE = 1.0 / math.sqrt(HS)
EXP_BIAS = -10.0 * math.log(2.0)   # exp(s*SCALE - 10ln2): keeps fp16 in range
SX = 16.0                 # fp8 quantization scale for x
SW = 1024.0               # fp8 quantization scale for Wq/Wk/Wv
INV_S = 1.0 / (SX * SW)   # folded into the projection evictions

P = 128
NT = T // 512          # 4 t-chunks of 512
CT = C // P            # 16 contraction tiles
JT = T // P            # 16 key tiles per head

_NC_CACHE = {}


def _build(phases=(1, 2, 3)):
    nc = bacc.Bacc(None, target_bir_lowering=False)

    xh8 = nc.declare_dram_parameter("xh8", [C, T], F8, isOutput=False)
    xh8r = nc.declare_dram_parameter("xh8r", [C, T], F8, isOutput=False)
    wq8 = nc.declare_dram_parameter("wq8", [C, M], F8, isOutput=False)
    wq8r = nc.declare_dram_parameter("wq8r", [C, M], F8, isOutput=False)
    wk8 = nc.declare_dram_parameter("wk8", [C, M], F8, isOutput=False)
    wk8r = nc.declare_dram_parameter("wk8r", [C, M], F8, isOutput=False)
    wv8 = nc.declare_dram_parameter("wv8", [C, M], F8, isOutput=False)
    wv8r = nc.declare_dram_parameter("wv8r", [C, M], F8, isOutput=False)
    woT = nc.declare_dram_parameter("woT", [M, C], F16, isOutput=False)
    bqc = nc.declare_dram_parameter("bqc", [P, H_LOC], F32, isOutput=False)
    bkc = nc.declare_dram_parameter("bkc", [P, H_LOC], F32, isOutput=False)
    bvr = nc.declare_dram_parameter("bvr", [1, M], F32, isOutput=False)
    ebias = nc.declare_dram_parameter("ebias", [P, 1], F32, isOutput=False)
    trimask = nc.declare_dram_parameter("trimask", [P, P], F16, isOutput=False)
    cosT = nc.declare_dram_parameter("cosT", [ROT, T], F16, isOutput=False)
    nsT = nc.declare_dram_parameter("nsT", [ROT, T], F16, isOutput=False)
    outT = nc.declare_dram_parameter("outT", [C, T], F16, isOutput=True)

    with tile.TileContext(nc) as tc, \
         tc.tile_pool(name="const", bufs=1) as const, \
         tc.tile_pool(name="xp", bufs=CT) as xpool, \
         tc.tile_pool(name="qc", bufs=2) as qpool, \
         tc.tile_pool(name="at", bufs=2) as atpool, \
         tc.tile_pool(name="rp", bufs=6) as rpool, \
         tc.tile_pool(name="exp", bufs=6) as expool, \
         tc.tile_pool(name="exs", bufs=2) as espool, \
         tc.tile_pool(name="rd", bufs=2) as rdpool, \
         tc.tile_pool(name="oe", bufs=3) as oepool, \
         tc.tile_pool(name="psA", bufs=3, space="PSUM") as psA, \
         tc.tile_pool(name="psS", bufs=3, space="PSUM") as psS, \
         tc.tile_pool(name="psV", bufs=2, space="PSUM") as psV:

        cos_sb = const.tile([ROT, T], F16, tag="cos")
        ns_sb = const.tile([ROT, T], F16, tag="ns")
        bq_sb = const.tile([P, H_LOC], F32, tag="bq")
        bk_sb = const.tile([P, H_LOC], F32, tag="bk")
        bv_sb = const.tile([1, M], F32, tag="bv")
        bvb_sb = const.tile([P, M], F32, tag="bvb")
        eb_sb = const.tile([P, 1], F32, tag="ebias")
        tri_sb = const.tile([P, P], F16, tag="trimask")
        k_res = const.tile([P, H_LOC, T], F16, tag="kres")
        v_res = const.tile([P, JT, M], F16, tag="vres")
        wq_t = [const.tile([P, CT, M], F8, tag=f"wq{i}", name=f"wq{i}")
                for i in range(2)]
        wk_t = [const.tile([P, CT, M], F8, tag=f"wk{i}", name=f"wk{i}")
                for i in range(2)]
        wv_t = [const.tile([P, CT, M], F8, tag=f"wv{i}", name=f"wv{i}")
                for i in range(2)]
        wo_b = const.tile([P, H_LOC, C], F16, tag="wob")
        wq_d = [wq8, wq8r]
        wk_d = [wk8, wk8r]
        wv_d = [wv8, wv8r]
        wre = [d[:].rearrange("(ct p) m -> p ct m", p=P)
               for d in (wq8, wq8r, wk8, wk8r, wv8, wv8r)]
        wqre, wqrre, wkre, wkrre, wvre, wvrre = wre
        wor = woT[:].rearrange("(mt p) c -> p mt c", p=P)

        xr = xh8[:].rearrange("(ct p) t -> p ct t", p=P)
        xrr = xh8r[:].rearrange("(ct p) t -> p ct t", p=P)
        otr = outT[:].rearrange("(co p) t -> p co t", p=P)

        def load_x(ic):
            ts0 = ic * 512
            xb = xpool.tile([P, CT, 512], F8, tag="xb8", name=f"xb{ic}", bufs=2)
            xbr = xpool.tile([P, CT, 512], F8, tag="xb8r", name=f"xbr{ic}",
                             bufs=2)
            for j in range(2):
                nc.sync.dma_start(out=xb[:, 8 * j:8 * j + 8, :],
                                  in_=xr[:, 8 * j:8 * j + 8, ts0:ts0 + 512])
                nc.sync.dma_start(out=xbr[:, 8 * j:8 * j + 8, :],
                                  in_=xrr[:, 8 * j:8 * j + 8, ts0:ts0 + 512])
            return xb, xbr

        # startup: x0 main+residual stream on the SP queue while wq pairs
        # stream on the Act queue; chunked so HWDGE overheads don't pace it.
        xb0 = xpool.tile([P, CT, 512], F8, tag="xb8", name="xb0", bufs=2)
        xb0r = xpool.tile([P, CT, 512], F8, tag="xb8r", name="xb0r", bufs=2)
        for j in range(4):
            a, b = 4 * j, 4 * j + 4
            nc.sync.dma_start(out=xb0[:, a:b, :], in_=xr[:, a:b, 0:512])
            nc.sync.dma_start(out=xb0r[:, a:b, :], in_=xrr[:, a:b, 0:512])
            nc.scalar.dma_start(out=wq_t[0][:, a:b, :], in_=wqre[:, a:b, :])
            nc.scalar.dma_start(out=wq_t[1][:, a:b, :], in_=wqrre[:, a:b, :])
        for j in range(2):
            a, b = 8 * j, 8 * j + 8
            nc.sync.dma_start(out=wv_t[0][:, a:b, :], in_=wvre[:, a:b, :])
        for j in range(2):
            a, b = 8 * j, 8 * j + 8
            nc.sync.dma_start(out=wv_t[1][:, a:b, :], in_=wvrre[:, a:b, :])
        for j in range(2):
            a, b = 8 * j, 8 * j + 8
            nc.sync.dma_start(out=wk_t[0][:, a:b, :], in_=wkre[:, a:b, :])
        for j in range(2):
            a, b = 8 * j, 8 * j + 8
            nc.sync.dma_start(out=wk_t[1][:, a:b, :], in_=wkrre[:, a:b, :])
        nc.gpsimd.dma_start(out=eb_sb[:], in_=ebias[:])
        nc.gpsimd.dma_start(out=tri_sb[:], in_=trimask[:])
        nc.gpsimd.dma_start(out=bq_sb[:], in_=bqc[:])
        nc.gpsimd.dma_start(out=bk_sb[:], in_=bkc[:])
        nc.gpsimd.dma_start(out=bv_sb[:], in_=bvr[:])
        nc.gpsimd.dma_start(out=cos_sb[:], in_=cosT[:])
        nc.gpsimd.dma_start(out=ns_sb[:], in_=nsT[:])
        nc.gpsimd.dma_start(out=bvb_sb[:], in_=bvr[0:1, :].to_broadcast([P, M]))

        NP = CT // 2   # 8 ct-pairs per contraction

        def fb_terms(w_t, xb, xbr):
            """(lhsT_tile, rhs_tile) per error-feedback term: main, w-res,
            x-res. All DoubleRow fp8 over ct-pairs."""
            return ((w_t[0], xb), (w_t[1], xb), (w_t[0], xbr))

        def rope_inplace(dst, tmp_src, ts0):
            """dst[0:ROT, 512] fp16 <- rope(tmp_src rows 0:ROT) in place.
            tmp_src rows are pre-rope biased values; dst may alias tmp_src."""
            sh = rpool.tile([ROT, 512], F16, tag="sh")
            nc.sync.dma_start(out=sh[0:HALF], in_=tmp_src[HALF:ROT])
            nc.sync.dma_start(out=sh[HALF:ROT], in_=tmp_src[0:HALF])
            rot = rpool.tile([ROT, 512], F16, tag="rot")
            nc.vector.tensor_tensor(rot[:], sh[:], ns_sb[:, ts0:ts0 + 512],
                                    mybir.AluOpType.mult)
            tcos = rpool.tile([ROT, 512], F16, tag="tcos")
            nc.vector.tensor_tensor(tcos[:], tmp_src[:ROT], cos_sb[:, ts0:ts0 + 512],
                                    mybir.AluOpType.mult)
            nc.vector.tensor_tensor(dst[0:ROT], tcos[:], rot[:],
                                    mybir.AluOpType.add)

        class Ph3:
            """Output projection for chunk ic; matmuls are dispensed one at a
            time (step) so they weave between attention matmuls."""

            def __init__(self, ic, attn, pools=None):
                self.ic = ic
                self.attn = attn
                self.items = [(co, mt) for co in range(CT) for mt in range(H_LOC)]
                self.pos = 0
                self.ps = None
                self.pools = pools or [psA]
                self.finishing = False

            def step(self, n=1):
                for _ in range(n):
                    if self.pos >= len(self.items):
                        return
                    co, mt = self.items[self.pos]
                    self.pos += 1
                    if mt == 0:
                        pool = self.pools[co % len(self.pools)]
                        self.ps = pool.tile([P, 512], F32,
                                            tag="psA" if pool is psA else "psS")
                    nc.tensor.matmul(
                        self.ps[:],
                        lhsT=wo_b[:, mt, co * P:(co + 1) * P],
                        rhs=self.attn[:, mt, :],
                        start=(mt == 0), stop=(mt == H_LOC - 1))
                    if mt == H_LOC - 1:
                        if co % 4 == 0:
                            self.ot = oepool.tile([P, 4, 512], F16, tag="ot")
                        if self.finishing and co % 2 == 1:
                            # post-attention block: DVE is draining attention
                            # tail work, so alternate evicts onto idle Act
                            nc.scalar.copy(self.ot[:, co % 4, :], self.ps[:])
                        else:
                            nc.vector.tensor_copy(out=self.ot[:, co % 4, :],
                                                  in_=self.ps[:])
                        last = self.ic == NT - 1
                        step = 2 if last else 4
                        if co % step == step - 1:
                            j0 = co % 4 - (step - 1)
                            nc.sync.dma_start(
                                out=otr[:, co - step + 1:co + 1,
                                        self.ic * 512:self.ic * 512 + 512],
                                in_=self.ot[:, j0:j0 + step, :])

            def finish(self):
                self.finishing = True
                self.step(len(self.items) - self.pos)

        pending = None

        for ic in range(NT):
            ts0 = ic * 512
            x_cur = (xb0, xb0r) if ic == 0 else x_next

            # ---- proj q ----
            qcur = qpool.tile([P, H_LOC, 512], F16, tag="qcur")
            xb8, xb8r = x_cur
            q_terms = fb_terms(wq_t, xb8, xb8r)
            if ic == 0:
                # pair-major with 4 concurrent PSUM groups: PE tracks the
                # x0/wq DMA chunk arrivals instead of stalling on one group
                ps_q = [psA.tile([P, 512], F32, tag="psA", name=f"psq{m}")
                        for m in range(3)]
                ps_q.append(psV.tile([P, 512], F32, tag="psV", name="psq3"))
                for a in range(NP):
                    for mt in range(H_LOC):
                        for ti, (wt, xt) in enumerate(q_terms):
                            nc.tensor.matmul(
                                ps_q[mt][:],
                                lhsT=wt[:, 2 * a:2 * a + 2,
                                        mt * P:(mt + 1) * P],
                                rhs=xt[:, 2 * a:2 * a + 2, :],
                                start=(a == 0 and ti == 0),
                                stop=(a == NP - 1 and ti == 2),
                                perf_mode=mybir.MatmulPerfMode.DoubleRow)
                for mt in range(H_LOC):
                    nc.scalar.activation(
                        qcur[:, mt, :], ps_q[mt][:],
                        mybir.ActivationFunctionType.Identity,
                        bias=bq_sb[:, mt:mt + 1], scale=INV_S)
                    rope_inplace(qcur[:, mt, :], qcur[:, mt, :], ts0)
            else:
                for mt in range(H_LOC):
                    ps = psA.tile([P, 512], F32, tag="psA")
                    for ti, (wt, xt) in enumerate(q_terms):
                        for a in range(NP):
                            nc.tensor.matmul(
                                ps[:],
                                lhsT=wt[:, 2 * a:2 * a + 2,
                                        mt * P:(mt + 1) * P],
                                rhs=xt[:, 2 * a:2 * a + 2, :],
                                start=(a == 0 and ti == 0),
                                stop=(a == NP - 1 and ti == 2),
                                perf_mode=mybir.MatmulPerfMode.DoubleRow)
                    nc.scalar.activation(
                        qcur[:, mt, :], ps[:],
                        mybir.ActivationFunctionType.Identity,
                        bias=bq_sb[:, mt:mt + 1], scale=INV_S)
                    rope_inplace(qcur[:, mt, :], qcur[:, mt, :], ts0)

            if ic == 0:
                for j in range(H_LOC):
                    nc.gpsimd.dma_start(out=wo_b[:, j:j + 1, :],
                                        in_=wor[:, j:j + 1, :])

            def proj_k():
                if ic == 0:
                    # w-residual term last: wk8r chunks are the last arrivals
                    k_terms = ((wk_t[0], xb8), (wk_t[0], xb8r), (wk_t[1], xb8))
                else:
                    k_terms = fb_terms(wk_t, xb8, xb8r)
                for mt in range(H_LOC):
                    ps = psA.tile([P, 512], F32, tag="psA")
                    for ti, (wt, xt) in enumerate(k_terms):
                        for a in range(NP):
                            nc.tensor.matmul(
                                ps[:],
                                lhsT=wt[:, 2 * a:2 * a + 2,
                                        mt * P:(mt + 1) * P],
                                rhs=xt[:, 2 * a:2 * a + 2, :],
                                start=(a == 0 and ti == 0),
                                stop=(a == NP - 1 and ti == 2),
                                perf_mode=mybir.MatmulPerfMode.DoubleRow)
                    nc.scalar.activation(
                        k_res[:, mt, ts0:ts0 + 512], ps[:],
                        mybir.ActivationFunctionType.Identity,
                        bias=bk_sb[:, mt:mt + 1], scale=INV_S)
                    rope_inplace(k_res[:, mt, ts0:ts0 + 512],
                                 k_res[:, mt, ts0:ts0 + 512], ts0)

            def proj_v():
                v_terms = ((xb8, wv_t[0]), (xb8, wv_t[1]), (xb8r, wv_t[0]))
                if ic == 0:
                    # pair-major on psS (idle before attention); term-major
                    # with the w-residual term last, since the residual
                    # weight chunks are the last DMAs to land
                    ps_v = [psS.tile([P, M], F32, tag="psS", name=f"psv{t}")
                            for t in range(3)]
                    ps_v.append(psV.tile([P, M], F32, tag="psV", name="psv3"))
                    vt0 = ((xb8, wv_t[0]), (xb8r, wv_t[0]), (xb8, wv_t[1]))
                    for ti, (xt, wt) in enumerate(vt0):
                        for a in range(NP):
                            for tt in range(4):
                                nc.tensor.matmul(
                                    ps_v[tt][:],
                                    lhsT=xt[:, 2 * a:2 * a + 2,
                                            tt * P:(tt + 1) * P],
                                    rhs=wt[:, 2 * a:2 * a + 2, :],
                                    start=(a == 0 and ti == 0),
                                    stop=(a == NP - 1 and ti == 2),
                                    perf_mode=mybir.MatmulPerfMode.DoubleRow)
                    for tt in range(4):
                        nc.vector.scalar_tensor_tensor(
                            out=v_res[:, 4 * ic + tt, :], in0=ps_v[tt][:],
                            scalar=INV_S, in1=bvb_sb[:],
                            op0=mybir.AluOpType.mult, op1=mybir.AluOpType.add)
                    return
                for tt in range(4):
                    ps = psA.tile([P, M], F32, tag="psA")
                    for ti, (xt, wt) in enumerate(v_terms):
                        for a in range(NP):
                            nc.tensor.matmul(
                                ps[:],
                                lhsT=xt[:, 2 * a:2 * a + 2,
                                        tt * P:(tt + 1) * P],
                                rhs=wt[:, 2 * a:2 * a + 2, :],
                                start=(a == 0 and ti == 0),
                                stop=(a == NP - 1 and ti == 2),
                                perf_mode=mybir.MatmulPerfMode.DoubleRow)
                    nc.vector.scalar_tensor_tensor(
                        out=v_res[:, 4 * ic + tt, :], in0=ps[:],
                        scalar=INV_S, in1=bvb_sb[:],
                        op0=mybir.AluOpType.mult, op1=mybir.AluOpType.add)

            if ic == 0:
                # wk lands last on the SP queue: fill the gap with proj v
                proj_v()
                proj_k()
            else:
                proj_k()
                proj_v()

            if ic + 1 < NT:
                x_next = load_x(ic + 1)

            # ---- attention for chunk ic (weaving ph3 of chunk ic-1) ----
            attn = atpool.tile([P, H_LOC, 512], F16, tag="attn")
            njt = 4 * ic + 4
            slots_left = H_LOC * njt
            for h in range(H_LOC):
                ps_av = psV.tile([P, 512], F32, tag="psV")
                exsum = espool.tile([P, 512], F16, tag="exsum")
                prev = None  # (ex tile, c0) awaiting its av matmul
                for jt in range(njt):
                    d = jt - 4 * ic
                    c0 = 128 * d if d > 0 else 0
                    ps_s = psS.tile([P, 512], F32, tag="psS")
                    nc.tensor.matmul(
                        ps_s[:, c0:],
                        lhsT=k_res[:, h, jt * P:(jt + 1) * P],
                        rhs=qcur[:, h, c0:],
                        start=True, stop=True)
                    ex = expool.tile([P, 512], F16, tag="ex")
                    nc.scalar.activation(
                        ex[:, c0:], ps_s[:, c0:],
                        mybir.ActivationFunctionType.Exp,
                        bias=eb_sb[:, 0:1], scale=SCALE)
                    if d >= 0:
                        # causal triangle: for every diagonal tile the global
                        # query base ts0+c0 equals the key base jt*P, so one
                        # [P,P] keep-where-col>=row mask serves them all
                        nc.vector.tensor_tensor(
                            ex[:, c0:c0 + P], ex[:, c0:c0 + P], tri_sb[:],
                            mybir.AluOpType.mult)
                    if pending is not None:
                        pending.step(1)
                    slots_left -= 1
                    if prev is not None:
                        pex, pc0, pjt = prev
                        nc.tensor.matmul(
                            ps_av[:, pc0:],
                            lhsT=v_res[:, pjt, h * HS:(h + 1) * HS],
                            rhs=pex[:, pc0:],
                            start=(pjt == 0), stop=False,
                            skip_group_check=True)
                    with nc.allow_low_precision(reason="fp16 softmax denom"):
                        if jt == 0:
                            if ic == 0:
                                nc.vector.tensor_copy(out=exsum[:], in_=ex[:])
                        elif jt == 1 and ic > 0:
                            nc.vector.tensor_tensor(
                                exsum[:], prev[0][:], ex[:],
                                mybir.AluOpType.add)
                        else:
                            nc.vector.tensor_tensor(
                                exsum[:, c0:], exsum[:, c0:], ex[:, c0:],
                                mybir.AluOpType.add)
                    prev = (ex, c0, jt)
                pex, pc0, pjt = prev
                nc.tensor.matmul(
                    ps_av[:, pc0:],
                    lhsT=v_res[:, pjt, h * HS:(h + 1) * HS],
                    rhs=pex[:, pc0:],
                    start=(pjt == 0), stop=True,
                    skip_group_check=True)
                rden = rdpool.tile([P, 512], F16, tag="rden")
                nc.gpsimd.partition_all_reduce(
                    rden[:], exsum[:], channels=P, reduce_op=bass_isa.ReduceOp.add)
                with nc.allow_low_precision(reason="softmax reciprocal"):
                    nc.vector.reciprocal(rden[:], rden[:])
                    nc.vector.tensor_tensor(
                        attn[:, h, :], ps_av[:], rden[:],
                        mybir.AluOpType.mult)

            if pending is not None:
                pending.finish()
            pending = Ph3(ic, attn,
                          pools=[psA, psS] if ic == NT - 1 else None)

        pending.finish()

    nc.finalize()
    return nc


def get_nc(phases=(1, 2, 3)):
    if phases not in _NC_CACHE:
        _NC_CACHE[phases] = _build(phases)
    return _NC_CACHE[phases]


def _rope_tables():
    inv_freq = 1.0 / (BASE ** (np.arange(0, ROT, 2, dtype=np.float64) / ROT))
    freqs = np.arange(T, dtype=np.float64)[:, None] * inv_freq[None, :]  # [T, 32]
    cos_h = np.cos(freqs).T.astype(np.float32)   # [32, T]
    sin_h = np.sin(freqs).T.astype(np.float32)
    cosT = np.concatenate([cos_h, cos_h], axis=0)          # [64, T]
    nsT = np.concatenate([-sin_h, sin_h], axis=0)          # [64, T] signed sin
    return (np.ascontiguousarray(cosT.astype(np.float16)),
            np.ascontiguousarray(nsT.astype(np.float16)))


def _split8(a, s):
    """a*s = a8 + a8r (both fp8 e4m3) up to second-order quantization."""
    scaled = a * np.float32(s)
    a8 = scaled.astype(NPF8)
    a8r = (scaled - a8.astype(np.float32)).astype(NPF8)
    return np.ascontiguousarray(a8), np.ascontiguousarray(a8r)


def make_in_maps(x, Wq, bq, Wk, bk, Wv, bv, Wo, bo):
    cosT, nsT = _rope_tables()
    in_maps = []
    for c in range(N_CORES):
        b, g = divmod(c, TPG)
        ms = slice(g * M, (g + 1) * M)
        xh8, xh8r = _split8(x[b].T, SX)
        wq8, wq8r = _split8(Wq[ms].T, SW)
        wk8, wk8r = _split8(Wk[ms].T, SW)
        wv8, wv8r = _split8(Wv[ms].T, SW)
        in_maps.append({
            "xh8": xh8, "xh8r": xh8r,
            "wq8": wq8, "wq8r": wq8r,
            "wk8": wk8, "wk8r": wk8r,
            "wv8": wv8, "wv8r": wv8r,
            "woT": np.ascontiguousarray(Wo[:, ms].T.astype(np.float16)),
            "bqc": np.ascontiguousarray(bq[ms].reshape(H_LOC, P).T),
            "bkc": np.ascontiguousarray(bk[ms].reshape(H_LOC, P).T),
            "bvr": np.ascontiguousarray(bv[ms].reshape(1, M)),
            "ebias": np.full((P, 1), EXP_BIAS, np.float32),
            "trimask": np.triu(np.ones((P, P), np.float16)),
            "cosT": cosT,
            "nsT": nsT,
        })
    return in_maps


def assemble(results, bo):
    out = np.empty((B, T, C), dtype=np.float32)
    for b in range(B):
        acc = results[b * TPG]["outT"].astype(np.float32)
        for g in range(1, TPG):
            acc = acc + results[b * TPG + g]["outT"].astype(np.float32)
        out[b] = acc.T + bo[None, :]
    return out


def kernel(x, Wq, bq, Wk, bk, Wv, bv, Wo, bo):
    nc = get_nc()
    in_maps = make_in_maps(np.asarray(x, np.float32),
                           np.asarray(Wq, np.float32), np.asarray(bq, np.float32),
                           np.asarray(Wk, np.float32), np.asarray(bk, np.float32),
                           np.asarray(Wv, np.float32), np.asarray(bv, np.float32),
                           np.asarray(Wo, np.float32), np.asarray(bo, np.float32))
    res = run_bass_kernel_spmd(nc, in_maps, list(range(N_CORES)))
    return assemble(res.results, np.asarray(bo, np.float32))
